# revision 14
# baseline (speedup 1.0000x reference)
"""Adaptive per-pixel Gaussian smoothing (7x7, sigma from a sigmoid of a
perspective map) on 8 Trainium2 NeuronCores — fp16 datapath revision.

Same structure as the fp32 baseline (data-parallel over (batch, H-half);
channel-major SBUF layout, ring-sum decomposition into 10 distinct-weight
rings) with three throughput changes:

1. fp16 everywhere on the DVE path (x, ring sums, weight maps, products):
   tensor_tensor runs in 2x_1p mode (2 elem/cycle/lane) instead of fp32 1x.
   To keep every operand 4B-aligned (required for 2x_1p), the x slab is
   DMA'd twice at column parities 0/1 (xwA/xwB) so every +-1/+-2/+-3 column
   shift reads from an even element offset. PSUM accumulation stays fp32.
2. The 10 weighted products are fused to one DVE op per ring (FD=2048).
3. Engine rebalance with a one-slab software pipeline: GpSimd (Pool)
   computes the three column sums R_b for slab s+1 while DVE runs slab
   s's rings and products (Pool's inputs are DMA-ready long in advance,
   and its outputs aren't needed until the next slab, so Pool stays off
   the critical path; slab 0's R_b run on the otherwise-idle DVE during
   the preamble). DMA issue lives on the sync (SP) sequencer; the
   PSUM->SBUF weight-broadcast copies are on ACT. Deep tile-pool
   buffering (xw 3, us/tm 4) decouples the DMA->DVE->PE->ACT pipeline;
   measured (cost model) DVE occupancy is ~88%.
"""

import numpy as np

import concourse.bass as bass
import concourse.tile as tile
from concourse import mybir
from concourse.bass_utils import run_bass_kernel_spmd

F32 = mybir.dt.float32
F16 = mybir.dt.float16
AF = mybir.ActivationFunctionType
OP = mybir.AluOpType

B, C, H, W = 4, 64, 256, 256
NCORES = 8
HS = H // 2          # 128 rows per core
G = 2                # row groups per core (partitions = G*64 channels)
GR = HS // G         # 64 rows per group
S = 8                # slab rows
NSLAB = GR // S      # 8 slabs
WP = W + 6           # 262 padded cols
WT = WP + 2          # 264-wide tiles so parity copy B fits at offset 1
LN2 = 0.6931471805599453

DS = [0, 1, 2, 4, 5, 8, 9, 10, 13, 18]
RING = {0: [(0, 0)], 1: [(0, 1), (1, 0)], 2: [(1, 1)], 4: [(0, 4), (4, 0)],
        5: [(1, 4), (4, 1)], 8: [(4, 4)], 9: [(0, 9), (9, 0)],
        10: [(1, 9), (9, 1)], 13: [(4, 9), (9, 4)], 18: [(9, 9)]}
POOL_D = ()          # ring chains all on DVE; Pool owns the column sums
DVE_D = [0, 4, 8, 1, 2, 5, 9, 10, 13, 18]   # ordered by R_b availability
PIPE_AFTER = 3       # emit next slab's R_b/Pool work after this many DVE rings

_CACHE = {}


def _build_nc():
    nc = bass.Bass()
    x_in = nc.declare_dram_parameter("x", [C, GR * G + 6, WP], F16, isOutput=False)
    p_in = nc.declare_dram_parameter("persp", [HS, W], F32, isOutput=False)
    abg_in = nc.declare_dram_parameter("abg", [128, 3], F32, isOutput=False)
    sels_in = nc.declare_dram_parameter("sels", [128, GR, 128], F16, isOutput=False)
    id_in = nc.declare_dram_parameter("ident", [128, 128], F16, isOutput=False)
    out_d = nc.declare_dram_parameter("out", [C, HS, W], F32, isOutput=True)

    H2 = S // 2
    Q = H2 // 2

    with tile.TileContext(nc) as tc:
        with (
            tc.tile_pool(name="const", bufs=1) as constp,
            tc.tile_pool(name="maps", bufs=1) as mapsp,
            tc.tile_pool(name="xw", bufs=3) as xwp,
            tc.tile_pool(name="rr", bufs=2) as rrp,
            tc.tile_pool(name="cd", bufs=2) as cdp,
            tc.tile_pool(name="us", bufs=4) as usp,
            tc.tile_pool(name="tm", bufs=4) as tmp_,
            tc.tile_pool(name="ob", bufs=2) as obp,
            tc.tile_pool(name="ps", bufs=2, space="PSUM") as psp,
            tc.tile_pool(name="pso", bufs=1, space="PSUM") as psop,
        ):
            # ---------- preamble: constants ----------
            # persp/abg first: the 16KB/partition sels DMA would otherwise
            # delay the ACT weight-map chain by ~7us at startup
            abg = constp.tile([128, 3], F32, tag="abg", name="abg")
            nc.sync.dma_start(abg[:], abg_in[:])

            persp = mapsp.tile([128, W], F32, tag="persp", name="persp_sb")
            nc.sync.dma_start(persp[:], p_in[:])

            nln2 = constp.tile([128, 1], F32, tag="nln2", name="nln2")
            nc.gpsimd.memset(nln2[:], -LN2)

            # ---------- preamble: per-pixel weight maps (pixel-major) ----------

            def mtile(tag, dt=F32):
                return mapsp.tile([128, W], dt, tag=tag, name=tag)

            sg = mtile("sg")
            nc.scalar.activation(sg[:], persp[:], AF.Sigmoid,
                                 bias=abg[:, 2:3], scale=abg[:, 1:2])
            sig = mtile("sig")
            nc.vector.tensor_scalar(sig[:], sg[:], abg[:, 0:1], 1e-4,
                                    OP.mult, OP.max)
            lg = mtile("lg")
            nc.scalar.activation(lg[:], sig[:], AF.Ln)
            tt = mtile("tt")
            nc.scalar.activation(tt[:], lg[:], AF.Exp, bias=nln2[:], scale=-2.0)
            e = {}
            e[1] = mtile("e1")
            nc.scalar.activation(e[1][:], tt[:], AF.Exp, scale=-1.0)
            for d, (i, j) in ((2, (1, 1)), (4, (2, 2)), (5, (4, 1)), (8, (4, 4)),
                              (9, (8, 1)), (10, (8, 2)), (13, (9, 4)), (18, (9, 9))):
                e[d] = mtile(f"e{d}")
                nc.gpsimd.tensor_mul(e[d][:], e[i][:], e[j][:])
            ssum = mtile("ssum")
            nc.gpsimd.tensor_add(ssum[:], e[1][:], e[4][:])
            nc.gpsimd.tensor_add(ssum[:], ssum[:], e[9][:])
            sv = mtile("sv")
            nc.gpsimd.tensor_scalar(sv[:], ssum[:], 2.0, 1.0, OP.mult, OP.add)
            l2 = mtile("l2")
            nc.scalar.activation(l2[:], sv[:], AF.Ln)
            u0f = mtile("u0f")
            nc.scalar.activation(u0f[:], l2[:], AF.Exp, scale=-2.0)
            # fp16 weight maps for the broadcast matmuls
            u = {}
            u[0] = mtile("u0", F16)
            nc.scalar.copy(u[0][:], u0f[:])
            for d in DS[1:]:
                u[d] = mtile(f"u{d}", F16)
                nc.gpsimd.tensor_mul(u[d][:], e[d][:], u0f[:])

            # ---------- slab-level helpers (state keyed per slab) ----------
            def dma_slab(s):
                """DMA both column-parity copies of the padded slab: image col
                c sits at col c+3 in xwA and c+4 in xwB so every shifted read
                lands on an even fp16 element (4B-aligned -> DVE 2x_1p)."""
                xwA = xwp.tile([128, S + 6, WT], F16, tag="xwA", name="xwA")
                xwB = xwp.tile([128, S + 6, WT], F16, tag="xwB", name="xwB")
                for g in range(G):
                    src = x_in[:, g * GR + s * S: g * GR + s * S + S + 6, :]
                    nc.sync.dma_start(xwA[64 * g:64 * (g + 1), :, 0:WP], src)
                    nc.sync.dma_start(xwB[64 * g:64 * (g + 1), :, 1:1 + WP], src)
                return xwA, xwB

            def col_sums(xwA, xwB, eng=None):
                """Symmetric column sums R_b[c] = x[c-r] + x[c+r] (DVE), valid
                cols 0..255, rows covering the +-3 halo."""
                Reng = eng or nc.gpsimd
                R = {}
                # ordered by first consumer (d4/d8 then d1 then d9)
                R[4] = rrp.tile([128, S + 6, W], F16, tag="R4", name="R4")
                Reng.tensor_add(R[4][:], xwB[:, :, 2:2 + W], xwB[:, :, 6:6 + W])
                R[1] = rrp.tile([128, S + 6, W], F16, tag="R1", name="R1")
                Reng.tensor_add(R[1][:], xwA[:, :, 2:2 + W], xwA[:, :, 4:4 + W])
                R[9] = rrp.tile([128, S + 6, W], F16, tag="R9", name="R9")
                Reng.tensor_add(R[9][:], xwA[:, :, 0:W], xwA[:, :, 6:6 + W])
                return R

            def center(xwB, rows):   # unshifted columns, row slice of the slab
                return xwB[:, rows, 4:4 + W]

            def pair_sum(d, R, tag):
                """The second (a,b) pair of ring d as a standalone add; its
                inputs are this slab's R_b (ready at slab start) and its
                consumer is the late cd combine, so it can run on Pool in the
                window before the next slab's column sums."""
                (a, b) = RING[d][1]
                ra = int(np.sqrt(a))
                tb = cdp.tile([128, S, W], F16, tag=tag, name=tag, bufs=2)
                nc.gpsimd.tensor_add(tb[:], R[b][:, 3 - ra:3 - ra + S, :],
                                     R[b][:, 3 + ra:3 + ra + S, :])
                return tb

            def ring_sum(d, R, xwB, eng, tag, bufs=1, tb_pre=None):
                """fp16 ring sum C_d computed on `eng`; returns the AP."""
                if d == 0:
                    return center(xwB, slice(3, 3 + S))
                cd = cdp.tile([128, S, W], F16, tag=tag, name=tag, bufs=bufs)
                first = True
                pend = None
                for pi, (a, b) in enumerate(RING[d]):
                    ra = int(np.sqrt(a))
                    if a == 0:
                        pend = R[b][:, 3:3 + S, :]
                        continue
                    if tb_pre is not None and pi == 1:
                        eng.tensor_add(cd[:], cd[:], tb_pre[:])
                        continue
                    hi = R[b][:, 3 - ra:3 - ra + S, :] if b else \
                        center(xwB, slice(3 - ra, 3 - ra + S))
                    lo = R[b][:, 3 + ra:3 + ra + S, :] if b else \
                        center(xwB, slice(3 + ra, 3 + ra + S))
                    if first:
                        eng.tensor_add(cd[:], hi, lo)
                        first = False
                    else:
                        tb = cdp.tile([128, S, W], F16, tag=f"{tag}_t",
                                      name=f"{tag}_t")
                        eng.tensor_add(tb[:], hi, lo)
                        eng.tensor_add(cd[:], cd[:], tb[:])
                if pend is not None:
                    eng.tensor_add(cd[:], cd[:], pend)
                return cd[:]

            def broadcast_us(s, d, tag):
                """u_d broadcast across the 128 (group, channel) partitions
                via selector matmuls into PSUM; ACT copies to fp16 SBUF."""
                us = usp.tile([128, S, W], F16, tag=tag, name=tag)
                for h in range(2):
                    ur = psp.tile([128, H2, W], F32, tag="urep", name="urep")
                    for r2 in range(H2):
                        row = s * S + h * H2 + r2
                        nc.tensor.matmul(ur[:, r2, :], sels[:, row, :],
                                         u[d][:], start=True, stop=True)
                    nc.scalar.copy(us[:, h * H2:(h + 1) * H2, :], ur[:])
                return us

            def emit_pool_side(s, R, xwB):
                """Next-slab Pool work: the d=5/13 ring chains (GpSimd)."""
                return {d: ring_sum(d, R, xwB, nc.gpsimd, f"cdP{d}", bufs=2)
                        for d in POOL_D}

            # ---------- prologue: slab 0's inputs and Pool-side work ----------
            xw = dma_slab(0)
            Rc = col_sums(*xw, eng=nc.vector)
            tbP = {d2: pair_sum(d2, Rc, f"tbP{d2}") for d2 in (13,)}

            # broadcast constants: not needed until the first selector matmul
            sels = constp.tile([128, GR, 128], F16, tag="sels", name="sels")
            nc.sync.dma_start(sels[:], sels_in[:])
            ident = constp.tile([128, 128], F16, tag="ident", name="ident")
            nc.sync.dma_start(ident[:], id_in[:])
            usP = {d: broadcast_us(0, d, f"usP{d}") for d in POOL_D}
            cdP = emit_pool_side(0, Rc, xw[1])

            # ---------- main loop over slabs ----------
            for s in range(NSLAB):
                xwA, xwB = xw
                R = Rc

                oacc = [psop.tile([128, H2, W], F32, tag=f"oacc{h}",
                                  name=f"oacc{h}") for h in range(2)]

                # d-sum accumulated by PE identity matmuls into PSUM. Products
                # are queued and emitted one ring late so the in-order PE queue
                # never head-of-line blocks on the DVE/Pool product it consumes.
                started = [False, False]
                pend_acc = []

                def flush_acc(last=False):
                    for k, tm in enumerate(pend_acc):
                        is_last_tm = last and k == len(pend_acc) - 1
                        for h in range(2):
                            for q in range(2):
                                rows = slice(h * H2 + q * Q, h * H2 + (q + 1) * Q)
                                nc.tensor.matmul(
                                    oacc[h][:, q * Q:(q + 1) * Q, :],
                                    ident[:],
                                    tm[:, rows, :],
                                    start=not started[h], stop=is_last_tm and q == 1,
                                    skip_group_check=True)
                            started[h] = True
                    pend_acc.clear()

                this_usP, this_cdP = usP, cdP

                late_acc = []
                this_tbP = tbP
                for di, d in enumerate(DVE_D):
                    us = broadcast_us(s, d, "us")
                    flush_acc()
                    cd_ap = ring_sum(d, R, xwB, nc.vector, "cd",
                                     tb_pre=this_tbP.get(d))
                    tm = tmp_.tile([128, S, W], F16, tag="tm", name="tm")
                    nc.vector.tensor_mul(tm[:], cd_ap, us[:])
                    pend_acc.append(tm)

                    if di == PIPE_AFTER and s + 1 < NSLAB:
                        # pipeline: next slab's inputs, column sums, and Pool
                        # chains are emitted here so Pool's work straddles the
                        # slab boundary and stays off the critical path
                        xw = dma_slab(s + 1)
                        Rc = col_sums(*xw)
                        usP = {d2: broadcast_us(s + 1, d2, f"usP{d2}")
                               for d2 in POOL_D}
                        cdP = emit_pool_side(s + 1, Rc, xw[1])
                        tbP = {d2: pair_sum(d2, Rc, f"tbP{d2}")
                               for d2 in (13,)}

                # Pool-side ring products on DVE at the end (chains long done)
                for d in POOL_D:
                    tmx = tmp_.tile([128, S, W], F16, tag=f"tmP{d}",
                                    name=f"tmP{d}")
                    nc.vector.tensor_mul(tmx[:], this_cdP[d], this_usP[d][:])
                    pend_acc.append(tmx)
                pend_acc.extend(late_acc)
                flush_acc(last=True)

                out_sb = obp.tile([128, S, W], F32, tag="ob", name="ob")
                for h in range(2):
                    nc.scalar.copy(out_sb[:, h * H2:(h + 1) * H2, :],
                                   oacc[h][:])
                for g in range(G):
                    nc.sync.dma_start(
                        out_d[:, g * GR + s * S: g * GR + s * S + S, :],
                        out_sb[64 * g:64 * (g + 1), :, :])
    return nc


def _selectors():
    """sels[k, i, m] = 1 iff pixel-row k feeds out partition m at row index i."""
    if "sels" not in _CACHE:
        sels = np.zeros((128, GR, 128), np.float16)
        for i in range(GR):
            sels[i, i, 0:64] = 1.0          # group 0: pixel row i
            sels[GR + i, i, 64:128] = 1.0   # group 1: pixel row 64+i
        _CACHE["sels"] = sels
    return _CACHE["sels"]


def _split_waits(nc):
    """Walrus on this toolchain accepts only one semaphore wait per compute
    instruction; hoist excess waits onto same-engine NoOps placed before."""
    for f in nc.m.functions:
        for bb in f.blocks:
            new_list = []
            for ins in bb.instructions:
                si = ins.sync_info
                if si is not None and len(si.on_wait) > 1:
                    waits = list(si.on_wait)
                    for k, w in enumerate(waits[:-1]):
                        nop = mybir.InstNoOp(name=f"{ins.name}-ws{k}",
                                             ins=[], outs=[])
                        nop.engine = ins.engine
                        nop.sync_info = mybir.SyncInfo(on_wait=[w], on_update=[])
                        new_list.append(nop)
                    ins.sync_info = mybir.SyncInfo(on_wait=[waits[-1]],
                                                  on_update=list(si.on_update))
                new_list.append(ins)
            bb.instructions = new_list


def _get_nc():
    if "nc" not in _CACHE:
        nc = _build_nc()
        _split_waits(nc)
        _CACHE["nc"] = nc
    return _CACHE["nc"]


def kernel(x, perspective, alpha, beta, gamma, kernel_size):
    assert int(kernel_size) == 7
    x = np.asarray(x, dtype=np.float32)
    perspective = np.asarray(perspective, dtype=np.float32)
    a = np.float32(np.asarray(alpha).reshape(-1)[0])
    bt = np.float32(np.asarray(beta).reshape(-1)[0])
    gm = np.float32(np.asarray(gamma).reshape(-1)[0])
    abg = np.broadcast_to(np.array([a, bt, gm], np.float32), (128, 3)).copy()
    sels = _selectors()
    ident = np.eye(128, dtype=np.float16)

    xp = np.pad(x.astype(np.float16), ((0, 0), (0, 0), (3, 3), (3, 3)))
    in_maps = []
    for b in range(B):
        for half in range(2):
            r0 = half * HS
            in_maps.append({
                "x": np.ascontiguousarray(xp[b, :, r0:r0 + HS + 6, :]),
                "persp": np.ascontiguousarray(perspective[b, 0, r0:r0 + HS, :]),
                "abg": abg,
                "sels": sels,
                "ident": ident,
            })

    nc = _get_nc()
    res = run_bass_kernel_spmd(nc, in_maps, list(range(NCORES)))
    _CACHE["last_res"] = res
    out = np.empty((B, C, H, W), np.float32)
    k = 0
    for b in range(B):
        for half in range(2):
            out[b, :, half * HS:(half + 1) * HS, :] = res.results[k]["out"]
            k += 1
    return out


if __name__ == "__main__":
    rng = np.random.default_rng(0)
    x = rng.standard_normal((B, C, H, W), dtype=np.float32)
    persp = rng.random((B, 1, H, W), dtype=np.float32)
    o = kernel(x=x, perspective=persp, alpha=np.ones(1, np.float32) * 3,
               beta=np.ones(1, np.float32), gamma=np.zeros(1, np.float32),
               kernel_size=7)
    print(o.shape, o.dtype, float(np.abs(o).mean()))


# revision 17
# speedup vs baseline: 1.0008x; 1.0008x over previous
"""Adaptive per-pixel Gaussian smoothing (7x7, sigma from a sigmoid of a
perspective map) on 8 Trainium2 NeuronCores — fp16 datapath revision.

Same structure as the fp32 baseline (data-parallel over (batch, H-half);
channel-major SBUF layout, ring-sum decomposition into 10 distinct-weight
rings) with three throughput changes:

1. fp16 everywhere on the DVE path (x, ring sums, weight maps, products):
   tensor_tensor runs in 2x_1p mode (2 elem/cycle/lane) instead of fp32 1x.
   To keep every operand 4B-aligned (required for 2x_1p), the x slab is
   DMA'd twice at column parities 0/1 (xwA/xwB) so every +-1/+-2/+-3 column
   shift reads from an even element offset. PSUM accumulation stays fp32.
2. The 10 weighted products are fused to one DVE op per ring (FD=2048).
3. Engine rebalance with a one-slab software pipeline: GpSimd (Pool)
   computes the three column sums R_b for slab s+1 while DVE runs slab
   s's rings and products (Pool's inputs are DMA-ready long in advance,
   and its outputs aren't needed until the next slab, so Pool stays off
   the critical path; slab 0's R_b run on the otherwise-idle DVE during
   the preamble). DMA issue lives on the sync (SP) sequencer; the
   PSUM->SBUF weight-broadcast copies are on ACT. Deep tile-pool
   buffering (xw 3, us/tm 4) decouples the DMA->DVE->PE->ACT pipeline;
   measured (cost model) DVE occupancy is ~88%.
"""

import numpy as np

import concourse.bass as bass
import concourse.tile as tile
from concourse import mybir
from concourse.bass_utils import run_bass_kernel_spmd

F32 = mybir.dt.float32
F16 = mybir.dt.float16
AF = mybir.ActivationFunctionType
OP = mybir.AluOpType

B, C, H, W = 4, 64, 256, 256
NCORES = 8
HS = H // 2          # 128 rows per core
G = 2                # row groups per core (partitions = G*64 channels)
GR = HS // G         # 64 rows per group
S = 8                # slab rows
NSLAB = GR // S      # 8 slabs
WP = W + 6           # 262 padded cols
WT = WP + 2          # 264-wide tiles so parity copy B fits at offset 1
LN2 = 0.6931471805599453

DS = [0, 1, 2, 4, 5, 8, 9, 10, 13, 18]
RING = {0: [(0, 0)], 1: [(0, 1), (1, 0)], 2: [(1, 1)], 4: [(0, 4), (4, 0)],
        5: [(1, 4), (4, 1)], 8: [(4, 4)], 9: [(0, 9), (9, 0)],
        10: [(1, 9), (9, 1)], 13: [(4, 9), (9, 4)], 18: [(9, 9)]}
POOL_D = ()          # ring chains all on DVE; Pool owns the column sums
DVE_D = [0, 4, 1, 8, 2, 5, 9, 10, 13, 18]   # ordered by R_b availability
PIPE_AFTER = 3       # emit next slab's R_b/Pool work after this many DVE rings

_CACHE = {}


def _build_nc():
    nc = bass.Bass()
    x_in = nc.declare_dram_parameter("x", [C, GR * G + 6, WP], F16, isOutput=False)
    p_in = nc.declare_dram_parameter("persp", [HS, W], F32, isOutput=False)
    abg_in = nc.declare_dram_parameter("abg", [128, 3], F32, isOutput=False)
    sels_in = nc.declare_dram_parameter("sels", [128, GR, 128], F16, isOutput=False)
    id_in = nc.declare_dram_parameter("ident", [128, 128], F16, isOutput=False)
    out_d = nc.declare_dram_parameter("out", [C, HS, W], F32, isOutput=True)

    H2 = S // 2
    Q = H2 // 2

    with tile.TileContext(nc) as tc:
        with (
            tc.tile_pool(name="const", bufs=1) as constp,
            tc.tile_pool(name="maps", bufs=1) as mapsp,
            tc.tile_pool(name="xw", bufs=3) as xwp,
            tc.tile_pool(name="rr", bufs=2) as rrp,
            tc.tile_pool(name="cd", bufs=2) as cdp,
            tc.tile_pool(name="us", bufs=4) as usp,
            tc.tile_pool(name="tm", bufs=4) as tmp_,
            tc.tile_pool(name="ob", bufs=2) as obp,
            tc.tile_pool(name="ps", bufs=2, space="PSUM") as psp,
            tc.tile_pool(name="pso", bufs=1, space="PSUM") as psop,
        ):
            # ---------- preamble: constants ----------
            # persp/abg first: the 16KB/partition sels DMA would otherwise
            # delay the ACT weight-map chain by ~7us at startup
            abg = constp.tile([128, 3], F32, tag="abg", name="abg")
            nc.sync.dma_start(abg[:], abg_in[:])

            persp = mapsp.tile([128, W], F32, tag="persp", name="persp_sb")
            nc.sync.dma_start(persp[:], p_in[:])

            nln2 = constp.tile([128, 1], F32, tag="nln2", name="nln2")
            nc.gpsimd.memset(nln2[:], -LN2)

            # ---------- preamble: per-pixel weight maps (pixel-major) ----------

            def mtile(tag, dt=F32):
                return mapsp.tile([128, W], dt, tag=tag, name=tag)

            sg = mtile("sg")
            nc.scalar.activation(sg[:], persp[:], AF.Sigmoid,
                                 bias=abg[:, 2:3], scale=abg[:, 1:2])
            sig = mtile("sig")
            nc.vector.tensor_scalar(sig[:], sg[:], abg[:, 0:1], 1e-4,
                                    OP.mult, OP.max)
            lg = mtile("lg")
            nc.scalar.activation(lg[:], sig[:], AF.Ln)
            tt = mtile("tt")
            nc.scalar.activation(tt[:], lg[:], AF.Exp, bias=nln2[:], scale=-2.0)
            e = {}
            e[1] = mtile("e1")
            nc.scalar.activation(e[1][:], tt[:], AF.Exp, scale=-1.0)
            for d, (i, j) in ((2, (1, 1)), (4, (2, 2)), (5, (4, 1)), (8, (4, 4)),
                              (9, (8, 1)), (10, (8, 2)), (13, (9, 4)), (18, (9, 9))):
                e[d] = mtile(f"e{d}")
                nc.gpsimd.tensor_mul(e[d][:], e[i][:], e[j][:])
            ssum = mtile("ssum")
            nc.gpsimd.tensor_add(ssum[:], e[1][:], e[4][:])
            nc.gpsimd.tensor_add(ssum[:], ssum[:], e[9][:])
            sv = mtile("sv")
            nc.gpsimd.tensor_scalar(sv[:], ssum[:], 2.0, 1.0, OP.mult, OP.add)
            l2 = mtile("l2")
            nc.scalar.activation(l2[:], sv[:], AF.Ln)
            u0f = mtile("u0f")
            nc.scalar.activation(u0f[:], l2[:], AF.Exp, scale=-2.0)
            # fp16 weight maps for the broadcast matmuls
            u = {}
            u[0] = mtile("u0", F16)
            nc.scalar.copy(u[0][:], u0f[:])
            for d in DS[1:]:
                u[d] = mtile(f"u{d}", F16)
                nc.gpsimd.tensor_mul(u[d][:], e[d][:], u0f[:])

            # ---------- slab-level helpers (state keyed per slab) ----------
            def dma_slab(s):
                """DMA both column-parity copies of the padded slab: image col
                c sits at col c+3 in xwA and c+4 in xwB so every shifted read
                lands on an even fp16 element (4B-aligned -> DVE 2x_1p)."""
                xwA = xwp.tile([128, S + 6, WT], F16, tag="xwA", name="xwA")
                xwB = xwp.tile([128, S + 6, WT], F16, tag="xwB", name="xwB")
                for g in range(G):
                    src = x_in[:, g * GR + s * S: g * GR + s * S + S + 6, :]
                    nc.sync.dma_start(xwA[64 * g:64 * (g + 1), :, 0:WP], src)
                    nc.sync.dma_start(xwB[64 * g:64 * (g + 1), :, 1:1 + WP], src)
                return xwA, xwB

            def col_sums(xwA, xwB, eng=None):
                """Symmetric column sums R_b[c] = x[c-r] + x[c+r] (DVE), valid
                cols 0..255, rows covering the +-3 halo."""
                Reng = eng or nc.gpsimd
                R = {}
                # ordered by first consumer (d4/d8 then d1 then d9)
                R[4] = rrp.tile([128, S + 6, W], F16, tag="R4", name="R4")
                Reng.tensor_add(R[4][:], xwB[:, :, 2:2 + W], xwB[:, :, 6:6 + W])
                R[1] = rrp.tile([128, S + 6, W], F16, tag="R1", name="R1")
                Reng.tensor_add(R[1][:], xwA[:, :, 2:2 + W], xwA[:, :, 4:4 + W])
                R[9] = rrp.tile([128, S + 6, W], F16, tag="R9", name="R9")
                Reng.tensor_add(R[9][:], xwA[:, :, 0:W], xwA[:, :, 6:6 + W])
                return R

            def center(xwB, rows):   # unshifted columns, row slice of the slab
                return xwB[:, rows, 4:4 + W]

            def pair_sum(d, R, tag):
                """The second (a,b) pair of ring d as a standalone add; its
                inputs are this slab's R_b (ready at slab start) and its
                consumer is the late cd combine, so it can run on Pool in the
                window before the next slab's column sums."""
                (a, b) = RING[d][1]
                ra = int(np.sqrt(a))
                tb = cdp.tile([128, S, W], F16, tag=tag, name=tag, bufs=2)
                nc.gpsimd.tensor_add(tb[:], R[b][:, 3 - ra:3 - ra + S, :],
                                     R[b][:, 3 + ra:3 + ra + S, :])
                return tb

            def ring_sum(d, R, xwB, eng, tag, bufs=1, tb_pre=None):
                """fp16 ring sum C_d computed on `eng`; returns the AP."""
                if d == 0:
                    return center(xwB, slice(3, 3 + S))
                cd = cdp.tile([128, S, W], F16, tag=tag, name=tag, bufs=bufs)
                first = True
                pend = None
                for pi, (a, b) in enumerate(RING[d]):
                    ra = int(np.sqrt(a))
                    if a == 0:
                        pend = R[b][:, 3:3 + S, :]
                        continue
                    if tb_pre is not None and pi == 1:
                        eng.tensor_add(cd[:], cd[:], tb_pre[:])
                        continue
                    hi = R[b][:, 3 - ra:3 - ra + S, :] if b else \
                        center(xwB, slice(3 - ra, 3 - ra + S))
                    lo = R[b][:, 3 + ra:3 + ra + S, :] if b else \
                        center(xwB, slice(3 + ra, 3 + ra + S))
                    if first:
                        eng.tensor_add(cd[:], hi, lo)
                        first = False
                    else:
                        tb = cdp.tile([128, S, W], F16, tag=f"{tag}_t",
                                      name=f"{tag}_t")
                        eng.tensor_add(tb[:], hi, lo)
                        eng.tensor_add(cd[:], cd[:], tb[:])
                if pend is not None:
                    eng.tensor_add(cd[:], cd[:], pend)
                return cd[:]

            def broadcast_us(s, d, tag):
                """u_d broadcast across the 128 (group, channel) partitions
                via selector matmuls into PSUM; ACT copies to fp16 SBUF."""
                us = usp.tile([128, S, W], F16, tag=tag, name=tag)
                for h in range(2):
                    ur = psp.tile([128, H2, W], F32, tag="urep", name="urep")
                    for r2 in range(H2):
                        row = s * S + h * H2 + r2
                        nc.tensor.matmul(ur[:, r2, :], sels[:, row, :],
                                         u[d][:], start=True, stop=True)
                    nc.scalar.copy(us[:, h * H2:(h + 1) * H2, :], ur[:])
                return us

            def emit_pool_side(s, R, xwB):
                """Next-slab Pool work: the d=5/13 ring chains (GpSimd)."""
                return {d: ring_sum(d, R, xwB, nc.gpsimd, f"cdP{d}", bufs=2)
                        for d in POOL_D}

            # ---------- prologue: slab 0's inputs and Pool-side work ----------
            xw = dma_slab(0)
            Rc = col_sums(*xw, eng=nc.vector)
            tbP = {d2: pair_sum(d2, Rc, f"tbP{d2}") for d2 in (13,)}

            # broadcast constants: not needed until the first selector matmul
            sels = constp.tile([128, GR, 128], F16, tag="sels", name="sels")
            nc.sync.dma_start(sels[:], sels_in[:])
            ident = constp.tile([128, 128], F16, tag="ident", name="ident")
            nc.sync.dma_start(ident[:], id_in[:])
            usP = {d: broadcast_us(0, d, f"usP{d}") for d in POOL_D}
            cdP = emit_pool_side(0, Rc, xw[1])

            # ---------- main loop over slabs ----------
            for s in range(NSLAB):
                xwA, xwB = xw
                R = Rc

                oacc = [psop.tile([128, H2, W], F32, tag=f"oacc{h}",
                                  name=f"oacc{h}") for h in range(2)]

                # d-sum accumulated by PE identity matmuls into PSUM. Products
                # are queued and emitted one ring late so the in-order PE queue
                # never head-of-line blocks on the DVE/Pool product it consumes.
                started = [False, False]
                pend_acc = []

                def flush_acc(last=False):
                    for k, tm in enumerate(pend_acc):
                        is_last_tm = last and k == len(pend_acc) - 1
                        for h in range(2):
                            for q in range(2):
                                rows = slice(h * H2 + q * Q, h * H2 + (q + 1) * Q)
                                nc.tensor.matmul(
                                    oacc[h][:, q * Q:(q + 1) * Q, :],
                                    ident[:],
                                    tm[:, rows, :],
                                    start=not started[h], stop=is_last_tm and q == 1,
                                    skip_group_check=True)
                            started[h] = True
                    pend_acc.clear()

                this_usP, this_cdP = usP, cdP

                late_acc = []
                this_tbP = tbP
                for di, d in enumerate(DVE_D):
                    us = broadcast_us(s, d, "us")
                    flush_acc()
                    cd_ap = ring_sum(d, R, xwB, nc.vector, "cd",
                                     tb_pre=this_tbP.get(d))
                    tm = tmp_.tile([128, S, W], F16, tag="tm", name="tm")
                    nc.vector.tensor_mul(tm[:], cd_ap, us[:])
                    pend_acc.append(tm)

                    if di == PIPE_AFTER and s + 1 < NSLAB:
                        # pipeline: next slab's inputs, column sums, and Pool
                        # chains are emitted here so Pool's work straddles the
                        # slab boundary and stays off the critical path
                        xw = dma_slab(s + 1)
                        Rc = col_sums(*xw)
                        usP = {d2: broadcast_us(s + 1, d2, f"usP{d2}")
                               for d2 in POOL_D}
                        cdP = emit_pool_side(s + 1, Rc, xw[1])
                        tbP = {d2: pair_sum(d2, Rc, f"tbP{d2}")
                               for d2 in (13,)}

                # Pool-side ring products on DVE at the end (chains long done)
                for d in POOL_D:
                    tmx = tmp_.tile([128, S, W], F16, tag=f"tmP{d}",
                                    name=f"tmP{d}")
                    nc.vector.tensor_mul(tmx[:], this_cdP[d], this_usP[d][:])
                    pend_acc.append(tmx)
                pend_acc.extend(late_acc)
                flush_acc(last=True)

                out_sb = obp.tile([128, S, W], F32, tag="ob", name="ob")
                for h in range(2):
                    nc.scalar.copy(out_sb[:, h * H2:(h + 1) * H2, :],
                                   oacc[h][:])
                for g in range(G):
                    nc.sync.dma_start(
                        out_d[:, g * GR + s * S: g * GR + s * S + S, :],
                        out_sb[64 * g:64 * (g + 1), :, :])
    return nc


def _selectors():
    """sels[k, i, m] = 1 iff pixel-row k feeds out partition m at row index i."""
    if "sels" not in _CACHE:
        sels = np.zeros((128, GR, 128), np.float16)
        for i in range(GR):
            sels[i, i, 0:64] = 1.0          # group 0: pixel row i
            sels[GR + i, i, 64:128] = 1.0   # group 1: pixel row 64+i
        _CACHE["sels"] = sels
    return _CACHE["sels"]


def _split_waits(nc):
    """Walrus on this toolchain accepts only one semaphore wait per compute
    instruction; hoist excess waits onto same-engine NoOps placed before."""
    for f in nc.m.functions:
        for bb in f.blocks:
            new_list = []
            for ins in bb.instructions:
                si = ins.sync_info
                if si is not None and len(si.on_wait) > 1:
                    waits = list(si.on_wait)
                    for k, w in enumerate(waits[:-1]):
                        nop = mybir.InstNoOp(name=f"{ins.name}-ws{k}",
                                             ins=[], outs=[])
                        nop.engine = ins.engine
                        nop.sync_info = mybir.SyncInfo(on_wait=[w], on_update=[])
                        new_list.append(nop)
                    ins.sync_info = mybir.SyncInfo(on_wait=[waits[-1]],
                                                  on_update=list(si.on_update))
                new_list.append(ins)
            bb.instructions = new_list


def _get_nc():
    if "nc" not in _CACHE:
        nc = _build_nc()
        _split_waits(nc)
        _CACHE["nc"] = nc
    return _CACHE["nc"]


def kernel(x, perspective, alpha, beta, gamma, kernel_size):
    assert int(kernel_size) == 7
    x = np.asarray(x, dtype=np.float32)
    perspective = np.asarray(perspective, dtype=np.float32)
    a = np.float32(np.asarray(alpha).reshape(-1)[0])
    bt = np.float32(np.asarray(beta).reshape(-1)[0])
    gm = np.float32(np.asarray(gamma).reshape(-1)[0])
    abg = np.broadcast_to(np.array([a, bt, gm], np.float32), (128, 3)).copy()
    sels = _selectors()
    ident = np.eye(128, dtype=np.float16)

    xp = np.pad(x.astype(np.float16), ((0, 0), (0, 0), (3, 3), (3, 3)))
    in_maps = []
    for b in range(B):
        for half in range(2):
            r0 = half * HS
            in_maps.append({
                "x": np.ascontiguousarray(xp[b, :, r0:r0 + HS + 6, :]),
                "persp": np.ascontiguousarray(perspective[b, 0, r0:r0 + HS, :]),
                "abg": abg,
                "sels": sels,
                "ident": ident,
            })

    nc = _get_nc()
    res = run_bass_kernel_spmd(nc, in_maps, list(range(NCORES)))
    _CACHE["last_res"] = res
    out = np.empty((B, C, H, W), np.float32)
    k = 0
    for b in range(B):
        for half in range(2):
            out[b, :, half * HS:(half + 1) * HS, :] = res.results[k]["out"]
            k += 1
    return out


if __name__ == "__main__":
    rng = np.random.default_rng(0)
    x = rng.standard_normal((B, C, H, W), dtype=np.float32)
    persp = rng.random((B, 1, H, W), dtype=np.float32)
    o = kernel(x=x, perspective=persp, alpha=np.ones(1, np.float32) * 3,
               beta=np.ones(1, np.float32), gamma=np.zeros(1, np.float32),
               kernel_size=7)
    print(o.shape, o.dtype, float(np.abs(o).mean()))


# revision 22
# speedup vs baseline: 1.0033x; 1.0026x over previous
"""Adaptive per-pixel Gaussian smoothing (7x7, sigma from a sigmoid of a
perspective map) on 8 Trainium2 NeuronCores — fp16 datapath revision.

Same structure as the fp32 baseline (data-parallel over (batch, H-half);
channel-major SBUF layout, ring-sum decomposition into 10 distinct-weight
rings) with three throughput changes:

1. fp16 everywhere on the DVE path (x, ring sums, weight maps, products):
   tensor_tensor runs in 2x_1p mode (2 elem/cycle/lane) instead of fp32 1x.
   To keep every operand 4B-aligned (required for 2x_1p), the x slab is
   DMA'd twice at column parities 0/1 (xwA/xwB) so every +-1/+-2/+-3 column
   shift reads from an even element offset. PSUM accumulation stays fp32.
2. The 10 weighted products are fused to one DVE op per ring (FD=2048).
3. Engine rebalance with a one-slab software pipeline: GpSimd (Pool)
   computes the three column sums R_b for slab s+1 while DVE runs slab
   s's rings and products (Pool's inputs are DMA-ready long in advance,
   and its outputs aren't needed until the next slab, so Pool stays off
   the critical path; slab 0's R_b run on the otherwise-idle DVE during
   the preamble). DMA issue lives on the sync (SP) sequencer; the
   PSUM->SBUF weight-broadcast copies are on ACT. Deep tile-pool
   buffering (xw 3, us/tm 4) decouples the DMA->DVE->PE->ACT pipeline;
   measured (cost model) DVE occupancy is ~88%.
"""

import numpy as np

import concourse.bass as bass
import concourse.tile as tile
from concourse import mybir
from concourse.bass_utils import run_bass_kernel_spmd

F32 = mybir.dt.float32
F16 = mybir.dt.float16
AF = mybir.ActivationFunctionType
OP = mybir.AluOpType

B, C, H, W = 4, 64, 256, 256
NCORES = 8
HS = H // 2          # 128 rows per core
G = 2                # row groups per core (partitions = G*64 channels)
GR = HS // G         # 64 rows per group
S = 8                # slab rows
NSLAB = GR // S      # 8 slabs
WP = W + 6           # 262 padded cols
WT = WP + 2          # 264-wide tiles so parity copy B fits at offset 1
LN2 = 0.6931471805599453

DS = [0, 1, 2, 4, 5, 8, 9, 10, 13, 18]
RING = {0: [(0, 0)], 1: [(0, 1), (1, 0)], 2: [(1, 1)], 4: [(0, 4), (4, 0)],
        5: [(1, 4), (4, 1)], 8: [(4, 4)], 9: [(0, 9), (9, 0)],
        10: [(1, 9), (9, 1)], 13: [(4, 9), (9, 4)], 18: [(9, 9)]}
POOL_D = ()          # ring chains all on DVE; Pool owns the column sums
DVE_D = [0, 4, 1, 8, 2, 5, 9, 10, 13, 18]   # ordered by R_b availability
PIPE_AFTER = 3       # emit next slab's R_b/Pool work after this many DVE rings

_CACHE = {}


def _build_nc():
    nc = bass.Bass()
    x_in = nc.declare_dram_parameter("x", [C, GR * G + 6, WP], F16, isOutput=False)
    p_in = nc.declare_dram_parameter("persp", [HS, W], F32, isOutput=False)
    abg_in = nc.declare_dram_parameter("abg", [128, 3], F32, isOutput=False)
    sels_in = nc.declare_dram_parameter("sels", [128, GR, 128], F16, isOutput=False)
    id_in = nc.declare_dram_parameter("ident", [128, 128], F16, isOutput=False)
    out_d = nc.declare_dram_parameter("out", [C, HS, W], F32, isOutput=True)

    H2 = S // 2
    Q = H2 // 2

    with tile.TileContext(nc) as tc:
        with (
            tc.tile_pool(name="const", bufs=1) as constp,
            tc.tile_pool(name="maps", bufs=1) as mapsp,
            tc.tile_pool(name="xw", bufs=3) as xwp,
            tc.tile_pool(name="rr", bufs=2) as rrp,
            tc.tile_pool(name="cd", bufs=2) as cdp,
            tc.tile_pool(name="us", bufs=4) as usp,
            tc.tile_pool(name="tm", bufs=4) as tmp_,
            tc.tile_pool(name="ob", bufs=2) as obp,
            tc.tile_pool(name="ps", bufs=2, space="PSUM") as psp,
            tc.tile_pool(name="pso", bufs=1, space="PSUM") as psop,
        ):
            # ---------- preamble: constants ----------
            # persp/abg first: the 16KB/partition sels DMA would otherwise
            # delay the ACT weight-map chain by ~7us at startup
            abg = constp.tile([128, 3], F32, tag="abg", name="abg")
            nc.sync.dma_start(abg[:], abg_in[:])

            persp = mapsp.tile([128, W], F32, tag="persp", name="persp_sb")
            nc.sync.dma_start(persp[:], p_in[:])

            nln2 = constp.tile([128, 1], F32, tag="nln2", name="nln2")
            nc.gpsimd.memset(nln2[:], -LN2)

            # ---------- preamble: per-pixel weight maps (pixel-major) ----------

            def mtile(tag, dt=F32):
                return mapsp.tile([128, W], dt, tag=tag, name=tag)

            sg = mtile("sg")
            nc.scalar.activation(sg[:], persp[:], AF.Sigmoid,
                                 bias=abg[:, 2:3], scale=abg[:, 1:2])
            sig = mtile("sig")
            nc.vector.tensor_scalar(sig[:], sg[:], abg[:, 0:1], 1e-4,
                                    OP.mult, OP.max)
            lg = mtile("lg")
            nc.scalar.activation(lg[:], sig[:], AF.Ln)
            tt = mtile("tt")
            nc.scalar.activation(tt[:], lg[:], AF.Exp, bias=nln2[:], scale=-2.0)
            e = {}
            e[1] = mtile("e1")
            nc.scalar.activation(e[1][:], tt[:], AF.Exp, scale=-1.0)
            for d, (i, j) in ((2, (1, 1)), (4, (2, 2)), (5, (4, 1)), (8, (4, 4)),
                              (9, (8, 1)), (10, (8, 2)), (13, (9, 4)), (18, (9, 9))):
                e[d] = mtile(f"e{d}")
                nc.gpsimd.tensor_mul(e[d][:], e[i][:], e[j][:])
            ssum = mtile("ssum")
            nc.gpsimd.tensor_add(ssum[:], e[1][:], e[4][:])
            nc.gpsimd.tensor_add(ssum[:], ssum[:], e[9][:])
            sv = mtile("sv")
            nc.gpsimd.tensor_scalar(sv[:], ssum[:], 2.0, 1.0, OP.mult, OP.add)
            l2 = mtile("l2")
            nc.scalar.activation(l2[:], sv[:], AF.Ln)
            u0f = mtile("u0f")
            nc.scalar.activation(u0f[:], l2[:], AF.Exp, scale=-2.0)
            # fp16 weight maps for the broadcast matmuls
            u = {}
            u[0] = mtile("u0", F16)
            nc.scalar.copy(u[0][:], u0f[:])
            for d in DS[1:]:
                u[d] = mtile(f"u{d}", F16)
                nc.gpsimd.tensor_mul(u[d][:], e[d][:], u0f[:])

            # ---------- slab-level helpers (state keyed per slab) ----------
            def dma_slab(s):
                """DMA both column-parity copies of the padded slab: image col
                c sits at col c+3 in xwA and c+4 in xwB so every shifted read
                lands on an even fp16 element (4B-aligned -> DVE 2x_1p)."""
                xwA = xwp.tile([128, S + 6, WT], F16, tag="xwA", name="xwA")
                xwB = xwp.tile([128, S + 6, WT], F16, tag="xwB", name="xwB")
                for g in range(G):
                    src = x_in[:, g * GR + s * S: g * GR + s * S + S + 6, :]
                    nc.sync.dma_start(xwA[64 * g:64 * (g + 1), :, 0:WP], src)
                    nc.sync.dma_start(xwB[64 * g:64 * (g + 1), :, 1:1 + WP], src)
                return xwA, xwB

            def col_sums(xwA, xwB, eng=None):
                """Symmetric column sums R_b[c] = x[c-r] + x[c+r] (DVE), valid
                cols 0..255, rows covering the +-3 halo."""
                Reng = eng or nc.gpsimd
                R = {}
                # ordered by first consumer (d4/d8 then d1 then d9)
                R[4] = rrp.tile([128, S + 6, W], F16, tag="R4", name="R4")
                Reng.tensor_add(R[4][:], xwB[:, :, 2:2 + W], xwB[:, :, 6:6 + W])
                R[1] = rrp.tile([128, S + 6, W], F16, tag="R1", name="R1")
                Reng.tensor_add(R[1][:], xwA[:, :, 2:2 + W], xwA[:, :, 4:4 + W])
                R[9] = rrp.tile([128, S + 6, W], F16, tag="R9", name="R9")
                Reng.tensor_add(R[9][:], xwA[:, :, 0:W], xwA[:, :, 6:6 + W])
                return R

            def center(xwB, rows):   # unshifted columns, row slice of the slab
                return xwB[:, rows, 4:4 + W]

            def pair_sum(d, R, tag):
                """The second (a,b) pair of ring d as a standalone add; its
                inputs are this slab's R_b (ready at slab start) and its
                consumer is the late cd combine, so it can run on Pool in the
                window before the next slab's column sums."""
                (a, b) = RING[d][1]
                ra = int(np.sqrt(a))
                tb = cdp.tile([128, S, W], F16, tag=tag, name=tag, bufs=2)
                nc.gpsimd.tensor_add(tb[:], R[b][:, 3 - ra:3 - ra + S, :],
                                     R[b][:, 3 + ra:3 + ra + S, :])
                return tb

            def ring_sum(d, R, xwB, eng, tag, bufs=1, tb_pre=None):
                """fp16 ring sum C_d computed on `eng`; returns the AP."""
                if d == 0:
                    return center(xwB, slice(3, 3 + S))
                cd = cdp.tile([128, S, W], F16, tag=tag, name=tag, bufs=bufs)
                first = True
                pend = None
                for pi, (a, b) in enumerate(RING[d]):
                    ra = int(np.sqrt(a))
                    if a == 0:
                        pend = R[b][:, 3:3 + S, :]
                        continue
                    if tb_pre is not None and pi == 1:
                        eng.tensor_add(cd[:], cd[:], tb_pre[:])
                        continue
                    hi = R[b][:, 3 - ra:3 - ra + S, :] if b else \
                        center(xwB, slice(3 - ra, 3 - ra + S))
                    lo = R[b][:, 3 + ra:3 + ra + S, :] if b else \
                        center(xwB, slice(3 + ra, 3 + ra + S))
                    if first:
                        eng.tensor_add(cd[:], hi, lo)
                        first = False
                    else:
                        tb = cdp.tile([128, S, W], F16, tag=f"{tag}_t",
                                      name=f"{tag}_t")
                        eng.tensor_add(tb[:], hi, lo)
                        eng.tensor_add(cd[:], cd[:], tb[:])
                if pend is not None:
                    eng.tensor_add(cd[:], cd[:], pend)
                return cd[:]

            def broadcast_us(s, d, tag):
                """u_d broadcast across the 128 (group, channel) partitions
                via selector matmuls into PSUM; ACT copies to fp16 SBUF."""
                us = usp.tile([128, S, W], F16, tag=tag, name=tag)
                for h in range(2):
                    ur = psp.tile([128, H2, W], F32, tag="urep", name="urep")
                    for r2 in range(H2):
                        row = s * S + h * H2 + r2
                        nc.tensor.matmul(ur[:, r2, :], sels[:, row, :],
                                         u[d][:], start=True, stop=True)
                    nc.scalar.copy(us[:, h * H2:(h + 1) * H2, :], ur[:])
                return us

            def emit_pool_side(s, R, xwB):
                """Next-slab Pool work: the d=5/13 ring chains (GpSimd)."""
                return {d: ring_sum(d, R, xwB, nc.gpsimd, f"cdP{d}", bufs=2)
                        for d in POOL_D}

            # ---------- prologue: slab 0's inputs and Pool-side work ----------
            xw = dma_slab(0)
            Rc = col_sums(*xw, eng=nc.vector)
            tbP = {d2: pair_sum(d2, Rc, f"tbP{d2}") for d2 in (13,)}

            # broadcast constants: not needed until the first selector matmul
            sels = constp.tile([128, GR, 128], F16, tag="sels", name="sels")
            nc.sync.dma_start(sels[:], sels_in[:])
            ident = constp.tile([128, 128], F16, tag="ident", name="ident")
            nc.sync.dma_start(ident[:], id_in[:])
            usP = {d: broadcast_us(0, d, f"usP{d}") for d in POOL_D}
            cdP = emit_pool_side(0, Rc, xw[1])

            # ---------- main loop over slabs ----------
            for s in range(NSLAB):
                xwA, xwB = xw
                R = Rc

                oacc = [psop.tile([128, H2, W], F32, tag=f"oacc{h}",
                                  name=f"oacc{h}") for h in range(2)]

                # d-sum accumulated by PE identity matmuls into PSUM. Products
                # are queued and emitted one ring late so the in-order PE queue
                # never head-of-line blocks on the DVE/Pool product it consumes.
                started = [False, False]
                pend_acc = []

                def flush_acc(last=False):
                    for k, tm in enumerate(pend_acc):
                        is_last_tm = last and k == len(pend_acc) - 1
                        for h in range(2):
                            for q in range(2):
                                rows = slice(h * H2 + q * Q, h * H2 + (q + 1) * Q)
                                nc.tensor.matmul(
                                    oacc[h][:, q * Q:(q + 1) * Q, :],
                                    ident[:],
                                    tm[:, rows, :],
                                    start=not started[h], stop=is_last_tm and q == 1,
                                    skip_group_check=True)
                            started[h] = True
                    pend_acc.clear()

                this_usP, this_cdP = usP, cdP

                late_acc = []
                this_tbP = tbP
                for di, d in enumerate(DVE_D):
                    us = broadcast_us(s, d, "us")
                    flush_acc()
                    cd_ap = ring_sum(d, R, xwB, nc.vector, "cd",
                                     tb_pre=this_tbP.get(d))
                    tm = tmp_.tile([128, S, W], F16, tag="tm", name="tm")
                    nc.vector.tensor_mul(tm[:], cd_ap, us[:])
                    pend_acc.append(tm)

                    if di == PIPE_AFTER and s + 1 < NSLAB:
                        # pipeline: next slab's inputs, column sums, and Pool
                        # chains are emitted here so Pool's work straddles the
                        # slab boundary and stays off the critical path
                        xw = dma_slab(s + 1)
                        Rc = col_sums(*xw)
                        usP = {d2: broadcast_us(s + 1, d2, f"usP{d2}")
                               for d2 in POOL_D}
                        cdP = emit_pool_side(s + 1, Rc, xw[1])
                        tbP = {d2: pair_sum(d2, Rc, f"tbP{d2}")
                               for d2 in (13,)}

                # Pool-side ring products on DVE at the end (chains long done)
                for d in POOL_D:
                    tmx = tmp_.tile([128, S, W], F16, tag=f"tmP{d}",
                                    name=f"tmP{d}")
                    nc.vector.tensor_mul(tmx[:], this_cdP[d], this_usP[d][:])
                    pend_acc.append(tmx)
                pend_acc.extend(late_acc)
                flush_acc(last=True)

                out_sb = obp.tile([128, S, W], F32, tag="ob", name="ob")
                for h in range(2):
                    nc.scalar.copy(out_sb[:, h * H2:(h + 1) * H2, :],
                                   oacc[h][:])
                # per-(g,h) output DMA: the h=0 half ships while PE/ACT are
                # still finishing h=1, shortening the final-slab tail
                for g in range(G):
                    for h in range(2):
                        r0 = s * S + h * H2
                        nc.sync.dma_start(
                            out_d[:, g * GR + r0: g * GR + r0 + H2, :],
                            out_sb[64 * g:64 * (g + 1),
                                   h * H2:(h + 1) * H2, :])
    return nc


def _selectors():
    """sels[k, i, m] = 1 iff pixel-row k feeds out partition m at row index i."""
    if "sels" not in _CACHE:
        sels = np.zeros((128, GR, 128), np.float16)
        for i in range(GR):
            sels[i, i, 0:64] = 1.0          # group 0: pixel row i
            sels[GR + i, i, 64:128] = 1.0   # group 1: pixel row 64+i
        _CACHE["sels"] = sels
    return _CACHE["sels"]


def _split_waits(nc):
    """Walrus on this toolchain accepts only one semaphore wait per compute
    instruction; hoist excess waits onto same-engine NoOps placed before."""
    for f in nc.m.functions:
        for bb in f.blocks:
            new_list = []
            for ins in bb.instructions:
                si = ins.sync_info
                if si is not None and len(si.on_wait) > 1:
                    waits = list(si.on_wait)
                    for k, w in enumerate(waits[:-1]):
                        nop = mybir.InstNoOp(name=f"{ins.name}-ws{k}",
                                             ins=[], outs=[])
                        nop.engine = ins.engine
                        nop.sync_info = mybir.SyncInfo(on_wait=[w], on_update=[])
                        new_list.append(nop)
                    ins.sync_info = mybir.SyncInfo(on_wait=[waits[-1]],
                                                  on_update=list(si.on_update))
                new_list.append(ins)
            bb.instructions = new_list


def _get_nc():
    if "nc" not in _CACHE:
        nc = _build_nc()
        _split_waits(nc)
        _CACHE["nc"] = nc
    return _CACHE["nc"]


def kernel(x, perspective, alpha, beta, gamma, kernel_size):
    assert int(kernel_size) == 7
    x = np.asarray(x, dtype=np.float32)
    perspective = np.asarray(perspective, dtype=np.float32)
    a = np.float32(np.asarray(alpha).reshape(-1)[0])
    bt = np.float32(np.asarray(beta).reshape(-1)[0])
    gm = np.float32(np.asarray(gamma).reshape(-1)[0])
    abg = np.broadcast_to(np.array([a, bt, gm], np.float32), (128, 3)).copy()
    sels = _selectors()
    ident = np.eye(128, dtype=np.float16)

    xp = np.pad(x.astype(np.float16), ((0, 0), (0, 0), (3, 3), (3, 3)))
    in_maps = []
    for b in range(B):
        for half in range(2):
            r0 = half * HS
            in_maps.append({
                "x": np.ascontiguousarray(xp[b, :, r0:r0 + HS + 6, :]),
                "persp": np.ascontiguousarray(perspective[b, 0, r0:r0 + HS, :]),
                "abg": abg,
                "sels": sels,
                "ident": ident,
            })

    nc = _get_nc()
    res = run_bass_kernel_spmd(nc, in_maps, list(range(NCORES)))
    _CACHE["last_res"] = res
    out = np.empty((B, C, H, W), np.float32)
    k = 0
    for b in range(B):
        for half in range(2):
            out[b, :, half * HS:(half + 1) * HS, :] = res.results[k]["out"]
            k += 1
    return out


if __name__ == "__main__":
    rng = np.random.default_rng(0)
    x = rng.standard_normal((B, C, H, W), dtype=np.float32)
    persp = rng.random((B, 1, H, W), dtype=np.float32)
    o = kernel(x=x, perspective=persp, alpha=np.ones(1, np.float32) * 3,
               beta=np.ones(1, np.float32), gamma=np.zeros(1, np.float32),
               kernel_size=7)
    print(o.shape, o.dtype, float(np.abs(o).mean()))


# revision 23
# speedup vs baseline: 1.0329x; 1.0294x over previous
"""Adaptive per-pixel Gaussian smoothing (7x7, sigma from a sigmoid of a
perspective map) on 8 Trainium2 NeuronCores — fp16 datapath revision.

Same structure as the fp32 baseline (data-parallel over (batch, H-half);
channel-major SBUF layout, ring-sum decomposition into 10 distinct-weight
rings) with three throughput changes:

1. fp16 everywhere on the DVE path (x, ring sums, weight maps, products):
   tensor_tensor runs in 2x_1p mode (2 elem/cycle/lane) instead of fp32 1x.
   To keep every operand 4B-aligned (required for 2x_1p), the x slab is
   DMA'd twice at column parities 0/1 (xwA/xwB) so every +-1/+-2/+-3 column
   shift reads from an even element offset. PSUM accumulation stays fp32.
2. The 10 weighted products are fused to one DVE op per ring (FD=2048).
3. Engine rebalance with a one-slab software pipeline: GpSimd (Pool)
   computes the three column sums R_b for slab s+1 while DVE runs slab
   s's rings and products (Pool's inputs are DMA-ready long in advance,
   and its outputs aren't needed until the next slab, so Pool stays off
   the critical path; slab 0's R_b run on the otherwise-idle DVE during
   the preamble). DMA issue lives on the sync (SP) sequencer; the
   PSUM->SBUF weight-broadcast copies are on ACT. Deep tile-pool
   buffering (xw 3, us/tm 4) decouples the DMA->DVE->PE->ACT pipeline;
   measured (cost model) DVE occupancy is ~88%.
"""

import numpy as np

import concourse.bass as bass
import concourse.tile as tile
from concourse import mybir
from concourse.bass_utils import run_bass_kernel_spmd

F32 = mybir.dt.float32
F16 = mybir.dt.float16
AF = mybir.ActivationFunctionType
OP = mybir.AluOpType

B, C, H, W = 4, 64, 256, 256
NCORES = 8
HS = H // 2          # 128 rows per core
G = 2                # row groups per core (partitions = G*64 channels)
GR = HS // G         # 64 rows per group
S = 8                # slab rows
NSLAB = GR // S      # 8 slabs
WP = W + 6           # 262 padded cols
WT = WP + 2          # 264-wide tiles so parity copy B fits at offset 1
LN2 = 0.6931471805599453

DS = [0, 1, 2, 4, 5, 8, 9, 10, 13, 18]
RING = {0: [(0, 0)], 1: [(0, 1), (1, 0)], 2: [(1, 1)], 4: [(0, 4), (4, 0)],
        5: [(1, 4), (4, 1)], 8: [(4, 4)], 9: [(0, 9), (9, 0)],
        10: [(1, 9), (9, 1)], 13: [(4, 9), (9, 4)], 18: [(9, 9)]}
POOL_D = ()          # ring chains all on DVE; Pool owns the column sums
DVE_D = [0, 4, 1, 8, 2, 5, 9, 10, 13, 18]
D_GROUPS = [(0,), (4, 1), (8, 2), (5, 9), (10, 13), (18,)]   # ordered by R_b availability
PIPE_AFTER = 3       # emit next slab's R_b/Pool work after this many DVE rings

_CACHE = {}


def _build_nc():
    nc = bass.Bass()
    x_in = nc.declare_dram_parameter("x", [C, GR * G + 6, WP], F16, isOutput=False)
    p_in = nc.declare_dram_parameter("persp", [HS, W], F32, isOutput=False)
    abg_in = nc.declare_dram_parameter("abg", [128, 3], F32, isOutput=False)
    sels_in = nc.declare_dram_parameter("sels", [128, GR, 128], F16, isOutput=False)
    id_in = nc.declare_dram_parameter("ident", [128, 128], F16, isOutput=False)
    out_d = nc.declare_dram_parameter("out", [C, HS, W], F32, isOutput=True)

    H2 = S // 2
    Q = H2 // 2

    with tile.TileContext(nc) as tc:
        with (
            tc.tile_pool(name="const", bufs=1) as constp,
            tc.tile_pool(name="maps", bufs=1) as mapsp,
            tc.tile_pool(name="xw", bufs=2) as xwp,
            tc.tile_pool(name="rr", bufs=2) as rrp,
            tc.tile_pool(name="cd", bufs=2) as cdp,
            tc.tile_pool(name="us", bufs=2) as usp,
            tc.tile_pool(name="tm", bufs=2) as tmp_,
            tc.tile_pool(name="ob", bufs=1) as obp,
            tc.tile_pool(name="ps", bufs=2, space="PSUM") as psp,
            tc.tile_pool(name="pso", bufs=1, space="PSUM") as psop,
        ):
            # ---------- preamble: constants ----------
            # persp/abg first: the 16KB/partition sels DMA would otherwise
            # delay the ACT weight-map chain by ~7us at startup
            abg = constp.tile([128, 3], F32, tag="abg", name="abg")
            nc.sync.dma_start(abg[:], abg_in[:])

            persp = mapsp.tile([128, W], F32, tag="persp", name="persp_sb")
            nc.sync.dma_start(persp[:], p_in[:])

            nln2 = constp.tile([128, 1], F32, tag="nln2", name="nln2")
            nc.gpsimd.memset(nln2[:], -LN2)

            # ---------- preamble: per-pixel weight maps (pixel-major) ----------

            def mtile(tag, dt=F32):
                return mapsp.tile([128, W], dt, tag=tag, name=tag)

            sg = mtile("sg")
            nc.scalar.activation(sg[:], persp[:], AF.Sigmoid,
                                 bias=abg[:, 2:3], scale=abg[:, 1:2])
            sig = mtile("sig")
            nc.vector.tensor_scalar(sig[:], sg[:], abg[:, 0:1], 1e-4,
                                    OP.mult, OP.max)
            lg = mtile("lg")
            nc.scalar.activation(lg[:], sig[:], AF.Ln)
            tt = mtile("tt")
            nc.scalar.activation(tt[:], lg[:], AF.Exp, bias=nln2[:], scale=-2.0)
            e = {}
            e[1] = mtile("e1")
            nc.scalar.activation(e[1][:], tt[:], AF.Exp, scale=-1.0)
            for d, (i, j) in ((2, (1, 1)), (4, (2, 2)), (5, (4, 1)), (8, (4, 4)),
                              (9, (8, 1)), (10, (8, 2)), (13, (9, 4)), (18, (9, 9))):
                e[d] = mtile(f"e{d}")
                nc.gpsimd.tensor_mul(e[d][:], e[i][:], e[j][:])
            ssum = mtile("ssum")
            nc.gpsimd.tensor_add(ssum[:], e[1][:], e[4][:])
            nc.gpsimd.tensor_add(ssum[:], ssum[:], e[9][:])
            sv = mtile("sv")
            nc.gpsimd.tensor_scalar(sv[:], ssum[:], 2.0, 1.0, OP.mult, OP.add)
            l2 = mtile("l2")
            nc.scalar.activation(l2[:], sv[:], AF.Ln)
            u0f = mtile("u0f")
            nc.scalar.activation(u0f[:], l2[:], AF.Exp, scale=-2.0)
            # fp16 weight maps for the broadcast matmuls
            u = {}
            u[0] = mtile("u0", F16)
            nc.scalar.copy(u[0][:], u0f[:])
            for d in DS[1:]:
                u[d] = mtile(f"u{d}", F16)
                nc.gpsimd.tensor_mul(u[d][:], e[d][:], u0f[:])

            # ---------- slab-level helpers (state keyed per slab) ----------
            def dma_slab(s):
                """DMA both column-parity copies of the padded slab: image col
                c sits at col c+3 in xwA and c+4 in xwB so every shifted read
                lands on an even fp16 element (4B-aligned -> DVE 2x_1p)."""
                xwA = xwp.tile([128, S + 6, WT], F16, tag="xwA", name="xwA")
                xwB = xwp.tile([128, S + 6, WT], F16, tag="xwB", name="xwB")
                for g in range(G):
                    src = x_in[:, g * GR + s * S: g * GR + s * S + S + 6, :]
                    nc.sync.dma_start(xwA[64 * g:64 * (g + 1), :, 0:WP], src)
                    nc.sync.dma_start(xwB[64 * g:64 * (g + 1), :, 1:1 + WP], src)
                return xwA, xwB

            def col_sums(xwA, xwB, eng=None):
                """Symmetric column sums R_b[c] = x[c-r] + x[c+r] (DVE), valid
                cols 0..255, rows covering the +-3 halo."""
                Reng = eng or nc.gpsimd
                R = {}
                # ordered by first consumer (d4/d8 then d1 then d9)
                R[4] = rrp.tile([128, S + 6, W], F16, tag="R4", name="R4")
                Reng.tensor_add(R[4][:], xwB[:, :, 2:2 + W], xwB[:, :, 6:6 + W])
                R[1] = rrp.tile([128, S + 6, W], F16, tag="R1", name="R1")
                Reng.tensor_add(R[1][:], xwA[:, :, 2:2 + W], xwA[:, :, 4:4 + W])
                R[9] = rrp.tile([128, S + 6, W], F16, tag="R9", name="R9")
                Reng.tensor_add(R[9][:], xwA[:, :, 0:W], xwA[:, :, 6:6 + W])
                return R

            def center(xwB, rows):   # unshifted columns, row slice of the slab
                return xwB[:, rows, 4:4 + W]

            def pair_sum(d, R, tag):
                """The second (a,b) pair of ring d as a standalone add; its
                inputs are this slab's R_b (ready at slab start) and its
                consumer is the late cd combine, so it can run on Pool in the
                window before the next slab's column sums."""
                (a, b) = RING[d][1]
                ra = int(np.sqrt(a))
                tb = cdp.tile([128, S, W], F16, tag=tag, name=tag, bufs=2)
                nc.gpsimd.tensor_add(tb[:], R[b][:, 3 - ra:3 - ra + S, :],
                                     R[b][:, 3 + ra:3 + ra + S, :])
                return tb

            def ring_sum(d, R, xwB, eng, tag, bufs=1, tb_pre=None,
                         into=None):
                """fp16 ring sum C_d computed on `eng`; returns the AP.
                `into=(tile, row0)` writes into a shared paired tile."""
                if d == 0:
                    return center(xwB, slice(3, 3 + S))
                if into is not None:
                    big, r0 = into
                    cd = big[:, r0:r0 + S, :]
                else:
                    cd = cdp.tile([128, S, W], F16, tag=tag, name=tag,
                                  bufs=bufs)
                first = True
                pend = None
                for pi, (a, b) in enumerate(RING[d]):
                    ra = int(np.sqrt(a))
                    if a == 0:
                        pend = R[b][:, 3:3 + S, :]
                        continue
                    if tb_pre is not None and pi == 1:
                        eng.tensor_add(cd[:], cd[:], tb_pre[:])
                        continue
                    hi = R[b][:, 3 - ra:3 - ra + S, :] if b else \
                        center(xwB, slice(3 - ra, 3 - ra + S))
                    lo = R[b][:, 3 + ra:3 + ra + S, :] if b else \
                        center(xwB, slice(3 + ra, 3 + ra + S))
                    if first:
                        eng.tensor_add(cd[:], hi, lo)
                        first = False
                    else:
                        tb = cdp.tile([128, S, W], F16, tag=f"{tag}_t",
                                      name=f"{tag}_t")
                        eng.tensor_add(tb[:], hi, lo)
                        eng.tensor_add(cd[:], cd[:], tb[:])
                if pend is not None:
                    eng.tensor_add(cd[:], cd[:], pend)
                return cd[:]

            def broadcast_us(s, d, tag, into=None):
                """u_d broadcast across the 128 (group, channel) partitions
                via selector matmuls into PSUM; ACT copies to fp16 SBUF.
                `into=(tile, row0)` writes into a shared paired tile."""
                if into is not None:
                    big, r0 = into
                    us = big[:, r0:r0 + S, :]
                else:
                    us = usp.tile([128, S, W], F16, tag=tag, name=tag)
                for h in range(2):
                    ur = psp.tile([128, H2, W], F32, tag="urep", name="urep")
                    for r2 in range(H2):
                        row = s * S + h * H2 + r2
                        nc.tensor.matmul(ur[:, r2, :], sels[:, row, :],
                                         u[d][:], start=True, stop=True)
                    nc.scalar.copy(us[:, h * H2:(h + 1) * H2, :], ur[:])
                return us

            def emit_pool_side(s, R, xwB):
                """Next-slab Pool work: the d=5/13 ring chains (GpSimd)."""
                return {d: ring_sum(d, R, xwB, nc.gpsimd, f"cdP{d}", bufs=2)
                        for d in POOL_D}

            # ---------- prologue: slab 0's inputs and Pool-side work ----------
            xw = dma_slab(0)
            Rc = col_sums(*xw, eng=nc.vector)
            tbP = {d2: pair_sum(d2, Rc, f"tbP{d2}") for d2 in (13,)}

            # broadcast constants: not needed until the first selector matmul
            sels = constp.tile([128, GR, 128], F16, tag="sels", name="sels")
            nc.sync.dma_start(sels[:], sels_in[:])
            ident = constp.tile([128, 128], F16, tag="ident", name="ident")
            nc.sync.dma_start(ident[:], id_in[:])
            usP = {d: broadcast_us(0, d, f"usP{d}") for d in POOL_D}
            cdP = emit_pool_side(0, Rc, xw[1])

            # ---------- main loop over slabs ----------
            for s in range(NSLAB):
                xwA, xwB = xw
                R = Rc

                oacc = [psop.tile([128, H2, W], F32, tag=f"oacc{h}",
                                  name=f"oacc{h}") for h in range(2)]

                # d-sum accumulated by PE identity matmuls into PSUM. Products
                # are queued and emitted one ring late so the in-order PE queue
                # never head-of-line blocks on the DVE/Pool product it consumes.
                started = [False, False]
                pend_acc = []

                def flush_acc(last=False):
                    for k, (tm, nparts) in enumerate(pend_acc):
                        is_last_tm = last and k == len(pend_acc) - 1
                        for p in range(nparts):
                            last_p = is_last_tm and p == nparts - 1
                            for h in range(2):
                                for q in range(2):
                                    rows = slice(p * S + h * H2 + q * Q,
                                                 p * S + h * H2 + (q + 1) * Q)
                                    nc.tensor.matmul(
                                        oacc[h][:, q * Q:(q + 1) * Q, :],
                                        ident[:],
                                        tm[:, rows, :],
                                        start=not started[h],
                                        stop=last_p and q == 1,
                                        skip_group_check=True)
                                started[h] = True
                    pend_acc.clear()

                this_usP, this_cdP = usP, cdP

                late_acc = []
                this_tbP = tbP
                for di, grp in enumerate(D_GROUPS):
                    flush_acc()
                    if len(grp) == 1:
                        d = grp[0]
                        us = broadcast_us(s, d, "us")
                        cd_ap = ring_sum(d, R, xwB, nc.vector, "cd",
                                         tb_pre=this_tbP.get(d))
                        tm = tmp_.tile([128, S, W], F16, tag="tm", name="tm")
                        nc.vector.tensor_mul(tm[:], cd_ap, us[:])
                        pend_acc.append((tm, 1))
                    else:
                        # paired rings: shared 16-row cd/us tiles, ONE product
                        # (halves the per-instruction dispatch/wait overhead)
                        us2 = usp.tile([128, 2 * S, W], F16, tag="us2",
                                       name="us2", bufs=2)
                        cd2 = cdp.tile([128, 2 * S, W], F16, tag="cd2",
                                       name="cd2", bufs=1)
                        for p, d in enumerate(grp):
                            broadcast_us(s, d, "us", into=(us2, p * S))
                            ring_sum(d, R, xwB, nc.vector, "cd",
                                     tb_pre=this_tbP.get(d),
                                     into=(cd2, p * S))
                        tm2 = tmp_.tile([128, 2 * S, W], F16, tag="tm2",
                                        name="tm2", bufs=2)
                        nc.vector.tensor_mul(tm2[:], cd2[:], us2[:])
                        pend_acc.append((tm2, 2))

                    if di == PIPE_AFTER and s + 1 < NSLAB:
                        # pipeline: next slab's inputs, column sums, and Pool
                        # chains are emitted here so Pool's work straddles the
                        # slab boundary and stays off the critical path
                        xw = dma_slab(s + 1)
                        Rc = col_sums(*xw)
                        usP = {d2: broadcast_us(s + 1, d2, f"usP{d2}")
                               for d2 in POOL_D}
                        cdP = emit_pool_side(s + 1, Rc, xw[1])
                        tbP = {d2: pair_sum(d2, Rc, f"tbP{d2}")
                               for d2 in (13,)}

                # Pool-side ring products on DVE at the end (chains long done)
                for d in POOL_D:
                    tmx = tmp_.tile([128, S, W], F16, tag=f"tmP{d}",
                                    name=f"tmP{d}")
                    nc.vector.tensor_mul(tmx[:], this_cdP[d], this_usP[d][:])
                    pend_acc.append((tmx, 1))
                pend_acc.extend((t, 1) for t in late_acc)
                flush_acc(last=True)

                out_sb = obp.tile([128, S, W], F32, tag="ob", name="ob")
                for h in range(2):
                    nc.scalar.copy(out_sb[:, h * H2:(h + 1) * H2, :],
                                   oacc[h][:])
                # per-(g,h) output DMA: the h=0 half ships while PE/ACT are
                # still finishing h=1, shortening the final-slab tail
                for g in range(G):
                    for h in range(2):
                        r0 = s * S + h * H2
                        nc.sync.dma_start(
                            out_d[:, g * GR + r0: g * GR + r0 + H2, :],
                            out_sb[64 * g:64 * (g + 1),
                                   h * H2:(h + 1) * H2, :])
    return nc


def _selectors():
    """sels[k, i, m] = 1 iff pixel-row k feeds out partition m at row index i."""
    if "sels" not in _CACHE:
        sels = np.zeros((128, GR, 128), np.float16)
        for i in range(GR):
            sels[i, i, 0:64] = 1.0          # group 0: pixel row i
            sels[GR + i, i, 64:128] = 1.0   # group 1: pixel row 64+i
        _CACHE["sels"] = sels
    return _CACHE["sels"]


def _split_waits(nc):
    """Walrus on this toolchain accepts only one semaphore wait per compute
    instruction; hoist excess waits onto same-engine NoOps placed before."""
    for f in nc.m.functions:
        for bb in f.blocks:
            new_list = []
            for ins in bb.instructions:
                si = ins.sync_info
                if si is not None and len(si.on_wait) > 1:
                    waits = list(si.on_wait)
                    for k, w in enumerate(waits[:-1]):
                        nop = mybir.InstNoOp(name=f"{ins.name}-ws{k}",
                                             ins=[], outs=[])
                        nop.engine = ins.engine
                        nop.sync_info = mybir.SyncInfo(on_wait=[w], on_update=[])
                        new_list.append(nop)
                    ins.sync_info = mybir.SyncInfo(on_wait=[waits[-1]],
                                                  on_update=list(si.on_update))
                new_list.append(ins)
            bb.instructions = new_list


def _get_nc():
    if "nc" not in _CACHE:
        nc = _build_nc()
        _split_waits(nc)
        _CACHE["nc"] = nc
    return _CACHE["nc"]


def kernel(x, perspective, alpha, beta, gamma, kernel_size):
    assert int(kernel_size) == 7
    x = np.asarray(x, dtype=np.float32)
    perspective = np.asarray(perspective, dtype=np.float32)
    a = np.float32(np.asarray(alpha).reshape(-1)[0])
    bt = np.float32(np.asarray(beta).reshape(-1)[0])
    gm = np.float32(np.asarray(gamma).reshape(-1)[0])
    abg = np.broadcast_to(np.array([a, bt, gm], np.float32), (128, 3)).copy()
    sels = _selectors()
    ident = np.eye(128, dtype=np.float16)

    xp = np.pad(x.astype(np.float16), ((0, 0), (0, 0), (3, 3), (3, 3)))
    in_maps = []
    for b in range(B):
        for half in range(2):
            r0 = half * HS
            in_maps.append({
                "x": np.ascontiguousarray(xp[b, :, r0:r0 + HS + 6, :]),
                "persp": np.ascontiguousarray(perspective[b, 0, r0:r0 + HS, :]),
                "abg": abg,
                "sels": sels,
                "ident": ident,
            })

    nc = _get_nc()
    res = run_bass_kernel_spmd(nc, in_maps, list(range(NCORES)))
    _CACHE["last_res"] = res
    out = np.empty((B, C, H, W), np.float32)
    k = 0
    for b in range(B):
        for half in range(2):
            out[b, :, half * HS:(half + 1) * HS, :] = res.results[k]["out"]
            k += 1
    return out


if __name__ == "__main__":
    rng = np.random.default_rng(0)
    x = rng.standard_normal((B, C, H, W), dtype=np.float32)
    persp = rng.random((B, 1, H, W), dtype=np.float32)
    o = kernel(x=x, perspective=persp, alpha=np.ones(1, np.float32) * 3,
               beta=np.ones(1, np.float32), gamma=np.zeros(1, np.float32),
               kernel_size=7)
    print(o.shape, o.dtype, float(np.abs(o).mean()))


# revision 24
# speedup vs baseline: 1.0436x; 1.0104x over previous
"""Adaptive per-pixel Gaussian smoothing (7x7, sigma from a sigmoid of a
perspective map) on 8 Trainium2 NeuronCores — fp16 datapath revision.

Same structure as the fp32 baseline (data-parallel over (batch, H-half);
channel-major SBUF layout, ring-sum decomposition into 10 distinct-weight
rings) with three throughput changes:

1. fp16 everywhere on the DVE path (x, ring sums, weight maps, products):
   tensor_tensor runs in 2x_1p mode (2 elem/cycle/lane) instead of fp32 1x.
   To keep every operand 4B-aligned (required for 2x_1p), the x slab is
   DMA'd twice at column parities 0/1 (xwA/xwB) so every +-1/+-2/+-3 column
   shift reads from an even element offset. PSUM accumulation stays fp32.
2. The 10 weighted products are fused to one DVE op per ring (FD=2048).
3. Engine rebalance with a one-slab software pipeline: GpSimd (Pool)
   computes the three column sums R_b for slab s+1 while DVE runs slab
   s's rings and products (Pool's inputs are DMA-ready long in advance,
   and its outputs aren't needed until the next slab, so Pool stays off
   the critical path; slab 0's R_b run on the otherwise-idle DVE during
   the preamble). DMA issue lives on the sync (SP) sequencer; the
   PSUM->SBUF weight-broadcast copies are on ACT. Deep tile-pool
   buffering (xw 3, us/tm 4) decouples the DMA->DVE->PE->ACT pipeline;
   measured (cost model) DVE occupancy is ~88%.
"""

import numpy as np

import concourse.bass as bass
import concourse.tile as tile
from concourse import mybir
from concourse.bass_utils import run_bass_kernel_spmd

F32 = mybir.dt.float32
F16 = mybir.dt.float16
AF = mybir.ActivationFunctionType
OP = mybir.AluOpType

B, C, H, W = 4, 64, 256, 256
NCORES = 8
HS = H // 2          # 128 rows per core
G = 2                # row groups per core (partitions = G*64 channels)
GR = HS // G         # 64 rows per group
S = 8                # slab rows
NSLAB = GR // S      # 8 slabs
WP = W + 6           # 262 padded cols
WT = WP + 2          # 264-wide tiles so parity copy B fits at offset 1
LN2 = 0.6931471805599453

DS = [0, 1, 2, 4, 5, 8, 9, 10, 13, 18]
RING = {0: [(0, 0)], 1: [(0, 1), (1, 0)], 2: [(1, 1)], 4: [(0, 4), (4, 0)],
        5: [(1, 4), (4, 1)], 8: [(4, 4)], 9: [(0, 9), (9, 0)],
        10: [(1, 9), (9, 1)], 13: [(4, 9), (9, 4)], 18: [(9, 9)]}
POOL_D = ()          # ring chains all on DVE; Pool owns the column sums
DVE_D = [0, 4, 1, 8, 2, 5, 9, 10, 13, 18]
D_GROUPS = [(0,), (4, 1), (8, 2), (5, 9), (10, 13), (18,)]   # ordered by R_b availability
PIPE_AFTER = 3       # emit next slab's R_b/Pool work after this many DVE rings

_CACHE = {}


def _build_nc():
    nc = bass.Bass()
    x_in = nc.declare_dram_parameter("x", [C, GR * G + 6, WP], F16, isOutput=False)
    p_in = nc.declare_dram_parameter("persp", [HS, W], F32, isOutput=False)
    abg_in = nc.declare_dram_parameter("abg", [128, 3], F32, isOutput=False)
    sels_in = nc.declare_dram_parameter("sels", [128, GR, 128], F16, isOutput=False)
    id_in = nc.declare_dram_parameter("ident", [128, 128], F16, isOutput=False)
    out_d = nc.declare_dram_parameter("out", [C, HS, W], F32, isOutput=True)

    H2 = S // 2
    Q = H2 // 2

    with tile.TileContext(nc) as tc:
        with (
            tc.tile_pool(name="const", bufs=1) as constp,
            tc.tile_pool(name="maps", bufs=1) as mapsp,
            tc.tile_pool(name="xw", bufs=2) as xwp,
            tc.tile_pool(name="rr", bufs=2) as rrp,
            tc.tile_pool(name="cd", bufs=2) as cdp,
            tc.tile_pool(name="us", bufs=2) as usp,
            tc.tile_pool(name="tm", bufs=2) as tmp_,
            tc.tile_pool(name="ob", bufs=1) as obp,
            tc.tile_pool(name="ps", bufs=2, space="PSUM") as psp,
            tc.tile_pool(name="pso", bufs=1, space="PSUM") as psop,
        ):
            # ---------- preamble: constants ----------
            # persp/abg first: the 16KB/partition sels DMA would otherwise
            # delay the ACT weight-map chain by ~7us at startup
            abg = constp.tile([128, 3], F32, tag="abg", name="abg")
            nc.sync.dma_start(abg[:], abg_in[:])

            persp = mapsp.tile([128, W], F32, tag="persp", name="persp_sb")
            nc.sync.dma_start(persp[:], p_in[:])

            nln2 = constp.tile([128, 1], F32, tag="nln2", name="nln2")
            nc.gpsimd.memset(nln2[:], -LN2)

            # ---------- preamble: per-pixel weight maps (pixel-major) ----------

            def mtile(tag, dt=F32):
                return mapsp.tile([128, W], dt, tag=tag, name=tag)

            sg = mtile("sg")
            nc.scalar.activation(sg[:], persp[:], AF.Sigmoid,
                                 bias=abg[:, 2:3], scale=abg[:, 1:2])
            sig = mtile("sig")
            nc.vector.tensor_scalar(sig[:], sg[:], abg[:, 0:1], 1e-4,
                                    OP.mult, OP.max)
            lg = mtile("sg")
            nc.scalar.activation(lg[:], sig[:], AF.Ln)
            tt = mtile("sig")
            nc.scalar.activation(tt[:], lg[:], AF.Exp, bias=nln2[:], scale=-2.0)
            e = {}
            e[1] = mtile("e1")
            nc.scalar.activation(e[1][:], tt[:], AF.Exp, scale=-1.0)
            for d, (i, j) in ((2, (1, 1)), (4, (2, 2)), (5, (4, 1)), (8, (4, 4)),
                              (9, (8, 1)), (10, (8, 2)), (13, (9, 4)), (18, (9, 9))):
                e[d] = mtile(f"e{d}")
                nc.gpsimd.tensor_mul(e[d][:], e[i][:], e[j][:])
            ssum = mtile("ssum")
            nc.gpsimd.tensor_add(ssum[:], e[1][:], e[4][:])
            nc.gpsimd.tensor_add(ssum[:], ssum[:], e[9][:])
            sv = mtile("sv")
            nc.gpsimd.tensor_scalar(sv[:], ssum[:], 2.0, 1.0, OP.mult, OP.add)
            l2 = mtile("l2")
            nc.scalar.activation(l2[:], sv[:], AF.Ln)
            u0f = mtile("u0f")
            nc.scalar.activation(u0f[:], l2[:], AF.Exp, scale=-2.0)
            # fp16 weight maps for the broadcast matmuls
            u = {}
            u[0] = mtile("u0", F16)
            nc.scalar.copy(u[0][:], u0f[:])
            for d in DS[1:]:
                u[d] = mtile(f"u{d}", F16)
                nc.gpsimd.tensor_mul(u[d][:], e[d][:], u0f[:])

            # ---------- slab-level helpers (state keyed per slab) ----------
            def dma_slab(s):
                """DMA both column-parity copies of the padded slab: image col
                c sits at col c+3 in xwA and c+4 in xwB so every shifted read
                lands on an even fp16 element (4B-aligned -> DVE 2x_1p)."""
                xwA = xwp.tile([128, S + 6, WT], F16, tag="xwA", name="xwA")
                xwB = xwp.tile([128, S + 6, WT], F16, tag="xwB", name="xwB")
                for g in range(G):
                    src = x_in[:, g * GR + s * S: g * GR + s * S + S + 6, :]
                    nc.sync.dma_start(xwA[64 * g:64 * (g + 1), :, 0:WP], src)
                    nc.sync.dma_start(xwB[64 * g:64 * (g + 1), :, 1:1 + WP], src)
                return xwA, xwB

            def col_sums(xwA, xwB, eng=None):
                """Symmetric column sums R_b[c] = x[c-r] + x[c+r] (DVE), valid
                cols 0..255, rows covering the +-3 halo."""
                Reng = eng or nc.gpsimd
                R = {}
                # ordered by first consumer (d4/d8 then d1 then d9)
                R[4] = rrp.tile([128, S + 6, W], F16, tag="R4", name="R4")
                Reng.tensor_add(R[4][:], xwB[:, :, 2:2 + W], xwB[:, :, 6:6 + W])
                R[1] = rrp.tile([128, S + 6, W], F16, tag="R1", name="R1")
                Reng.tensor_add(R[1][:], xwA[:, :, 2:2 + W], xwA[:, :, 4:4 + W])
                R[9] = rrp.tile([128, S + 6, W], F16, tag="R9", name="R9")
                Reng.tensor_add(R[9][:], xwA[:, :, 0:W], xwA[:, :, 6:6 + W])
                return R

            def center(xwB, rows):   # unshifted columns, row slice of the slab
                return xwB[:, rows, 4:4 + W]

            def pair_sum(d, R, tag):
                """The second (a,b) pair of ring d as a standalone add; its
                inputs are this slab's R_b (ready at slab start) and its
                consumer is the late cd combine, so it can run on Pool in the
                window before the next slab's column sums."""
                (a, b) = RING[d][1]
                ra = int(np.sqrt(a))
                tb = cdp.tile([128, S, W], F16, tag=tag, name=tag, bufs=2)
                nc.gpsimd.tensor_add(tb[:], R[b][:, 3 - ra:3 - ra + S, :],
                                     R[b][:, 3 + ra:3 + ra + S, :])
                return tb

            def ring_sum(d, R, xwB, eng, tag, bufs=1, tb_pre=None,
                         into=None):
                """fp16 ring sum C_d computed on `eng`; returns the AP.
                `into=(tile, row0)` writes into a shared paired tile."""
                if d == 0:
                    return center(xwB, slice(3, 3 + S))
                if into is not None:
                    big, r0 = into
                    cd = big[:, r0:r0 + S, :]
                else:
                    cd = cdp.tile([128, S, W], F16, tag=tag, name=tag,
                                  bufs=bufs)
                first = True
                pend = None
                for pi, (a, b) in enumerate(RING[d]):
                    ra = int(np.sqrt(a))
                    if a == 0:
                        pend = R[b][:, 3:3 + S, :]
                        continue
                    if tb_pre is not None and pi == 1:
                        eng.tensor_add(cd[:], cd[:], tb_pre[:])
                        continue
                    hi = R[b][:, 3 - ra:3 - ra + S, :] if b else \
                        center(xwB, slice(3 - ra, 3 - ra + S))
                    lo = R[b][:, 3 + ra:3 + ra + S, :] if b else \
                        center(xwB, slice(3 + ra, 3 + ra + S))
                    if first:
                        eng.tensor_add(cd[:], hi, lo)
                        first = False
                    else:
                        tb = cdp.tile([128, S, W], F16, tag=f"{tag}_t",
                                      name=f"{tag}_t")
                        eng.tensor_add(tb[:], hi, lo)
                        eng.tensor_add(cd[:], cd[:], tb[:])
                if pend is not None:
                    eng.tensor_add(cd[:], cd[:], pend)
                return cd[:]

            def broadcast_us(s, d, tag, into=None):
                """u_d broadcast across the 128 (group, channel) partitions
                via selector matmuls into PSUM; ACT copies to fp16 SBUF.
                `into=(tile, row0)` writes into a shared paired tile."""
                if into is not None:
                    big, r0 = into
                    us = big[:, r0:r0 + S, :]
                else:
                    us = usp.tile([128, S, W], F16, tag=tag, name=tag)
                for h in range(2):
                    ur = psp.tile([128, H2, W], F32, tag="urep", name="urep")
                    for r2 in range(H2):
                        row = s * S + h * H2 + r2
                        nc.tensor.matmul(ur[:, r2, :], sels[:, row, :],
                                         u[d][:], start=True, stop=True)
                    nc.scalar.copy(us[:, h * H2:(h + 1) * H2, :], ur[:])
                return us

            def emit_pool_side(s, R, xwB):
                """Next-slab Pool work: the d=5/13 ring chains (GpSimd)."""
                return {d: ring_sum(d, R, xwB, nc.gpsimd, f"cdP{d}", bufs=2)
                        for d in POOL_D}

            # ---------- prologue: slab 0's inputs and Pool-side work ----------
            xw = dma_slab(0)
            Rc = col_sums(*xw, eng=nc.vector)
            tbP = {d2: pair_sum(d2, Rc, f"tbP{d2}") for d2 in (13,)}

            # broadcast constants: not needed until the first selector matmul
            sels = constp.tile([128, GR, 128], F16, tag="sels", name="sels")
            nc.sync.dma_start(sels[:], sels_in[:])
            ident = constp.tile([128, 128], F16, tag="ident", name="ident")
            nc.sync.dma_start(ident[:], id_in[:])
            usP = {d: broadcast_us(0, d, f"usP{d}") for d in POOL_D}
            cdP = emit_pool_side(0, Rc, xw[1])

            # ---------- main loop over slabs ----------
            for s in range(NSLAB):
                xwA, xwB = xw
                R = Rc

                oacc = [psop.tile([128, H2, W], F32, tag=f"oacc{h}",
                                  name=f"oacc{h}") for h in range(2)]

                # d-sum accumulated by PE identity matmuls into PSUM. Products
                # are queued and emitted one ring late so the in-order PE queue
                # never head-of-line blocks on the DVE/Pool product it consumes.
                started = [False, False]
                pend_acc = []

                def flush_acc(last=False):
                    for k, (tm, nparts) in enumerate(pend_acc):
                        is_last_tm = last and k == len(pend_acc) - 1
                        for p in range(nparts):
                            last_p = is_last_tm and p == nparts - 1
                            for h in range(2):
                                for q in range(2):
                                    rows = slice(p * S + h * H2 + q * Q,
                                                 p * S + h * H2 + (q + 1) * Q)
                                    nc.tensor.matmul(
                                        oacc[h][:, q * Q:(q + 1) * Q, :],
                                        ident[:],
                                        tm[:, rows, :],
                                        start=not started[h],
                                        stop=last_p and q == 1,
                                        skip_group_check=True)
                                started[h] = True
                    pend_acc.clear()

                this_usP, this_cdP = usP, cdP

                late_acc = []
                this_tbP = tbP
                for di, grp in enumerate(D_GROUPS):
                    flush_acc()
                    if len(grp) == 1:
                        d = grp[0]
                        us = broadcast_us(s, d, "us")
                        cd_ap = ring_sum(d, R, xwB, nc.vector, "cd",
                                         tb_pre=this_tbP.get(d))
                        tm = tmp_.tile([128, S, W], F16, tag="tm", name="tm")
                        nc.vector.tensor_mul(tm[:], cd_ap, us[:])
                        pend_acc.append((tm, 1))
                    else:
                        # paired rings: shared 16-row cd/us tiles, ONE product
                        # (halves the per-instruction dispatch/wait overhead)
                        us2 = usp.tile([128, 2 * S, W], F16, tag="us2",
                                       name="us2", bufs=3)
                        cd2 = cdp.tile([128, 2 * S, W], F16, tag="cd2",
                                       name="cd2", bufs=1)
                        for p, d in enumerate(grp):
                            broadcast_us(s, d, "us", into=(us2, p * S))
                            ring_sum(d, R, xwB, nc.vector, "cd",
                                     tb_pre=this_tbP.get(d),
                                     into=(cd2, p * S))
                        tm2 = tmp_.tile([128, 2 * S, W], F16, tag="tm2",
                                        name="tm2", bufs=2)
                        nc.vector.tensor_mul(tm2[:], cd2[:], us2[:])
                        pend_acc.append((tm2, 2))

                    if di == PIPE_AFTER and s + 1 < NSLAB:
                        # pipeline: next slab's inputs, column sums, and Pool
                        # chains are emitted here so Pool's work straddles the
                        # slab boundary and stays off the critical path
                        xw = dma_slab(s + 1)
                        Rc = col_sums(*xw)
                        usP = {d2: broadcast_us(s + 1, d2, f"usP{d2}")
                               for d2 in POOL_D}
                        cdP = emit_pool_side(s + 1, Rc, xw[1])
                        tbP = {d2: pair_sum(d2, Rc, f"tbP{d2}")
                               for d2 in (13,)}

                # Pool-side ring products on DVE at the end (chains long done)
                for d in POOL_D:
                    tmx = tmp_.tile([128, S, W], F16, tag=f"tmP{d}",
                                    name=f"tmP{d}")
                    nc.vector.tensor_mul(tmx[:], this_cdP[d], this_usP[d][:])
                    pend_acc.append((tmx, 1))
                pend_acc.extend((t, 1) for t in late_acc)
                flush_acc(last=True)

                out_sb = obp.tile([128, S, W], F32, tag="ob", name="ob")
                for h in range(2):
                    nc.scalar.copy(out_sb[:, h * H2:(h + 1) * H2, :],
                                   oacc[h][:])
                # per-(g,h) output DMA: the h=0 half ships while PE/ACT are
                # still finishing h=1, shortening the final-slab tail
                for g in range(G):
                    for h in range(2):
                        r0 = s * S + h * H2
                        nc.sync.dma_start(
                            out_d[:, g * GR + r0: g * GR + r0 + H2, :],
                            out_sb[64 * g:64 * (g + 1),
                                   h * H2:(h + 1) * H2, :])
    return nc


def _selectors():
    """sels[k, i, m] = 1 iff pixel-row k feeds out partition m at row index i."""
    if "sels" not in _CACHE:
        sels = np.zeros((128, GR, 128), np.float16)
        for i in range(GR):
            sels[i, i, 0:64] = 1.0          # group 0: pixel row i
            sels[GR + i, i, 64:128] = 1.0   # group 1: pixel row 64+i
        _CACHE["sels"] = sels
    return _CACHE["sels"]


def _split_waits(nc):
    """Walrus on this toolchain accepts only one semaphore wait per compute
    instruction; hoist excess waits onto same-engine NoOps placed before."""
    for f in nc.m.functions:
        for bb in f.blocks:
            new_list = []
            for ins in bb.instructions:
                si = ins.sync_info
                if si is not None and len(si.on_wait) > 1:
                    waits = list(si.on_wait)
                    for k, w in enumerate(waits[:-1]):
                        nop = mybir.InstNoOp(name=f"{ins.name}-ws{k}",
                                             ins=[], outs=[])
                        nop.engine = ins.engine
                        nop.sync_info = mybir.SyncInfo(on_wait=[w], on_update=[])
                        new_list.append(nop)
                    ins.sync_info = mybir.SyncInfo(on_wait=[waits[-1]],
                                                  on_update=list(si.on_update))
                new_list.append(ins)
            bb.instructions = new_list


def _get_nc():
    if "nc" not in _CACHE:
        nc = _build_nc()
        _split_waits(nc)
        _CACHE["nc"] = nc
    return _CACHE["nc"]


def kernel(x, perspective, alpha, beta, gamma, kernel_size):
    assert int(kernel_size) == 7
    x = np.asarray(x, dtype=np.float32)
    perspective = np.asarray(perspective, dtype=np.float32)
    a = np.float32(np.asarray(alpha).reshape(-1)[0])
    bt = np.float32(np.asarray(beta).reshape(-1)[0])
    gm = np.float32(np.asarray(gamma).reshape(-1)[0])
    abg = np.broadcast_to(np.array([a, bt, gm], np.float32), (128, 3)).copy()
    sels = _selectors()
    ident = np.eye(128, dtype=np.float16)

    xp = np.pad(x.astype(np.float16), ((0, 0), (0, 0), (3, 3), (3, 3)))
    in_maps = []
    for b in range(B):
        for half in range(2):
            r0 = half * HS
            in_maps.append({
                "x": np.ascontiguousarray(xp[b, :, r0:r0 + HS + 6, :]),
                "persp": np.ascontiguousarray(perspective[b, 0, r0:r0 + HS, :]),
                "abg": abg,
                "sels": sels,
                "ident": ident,
            })

    nc = _get_nc()
    res = run_bass_kernel_spmd(nc, in_maps, list(range(NCORES)))
    _CACHE["last_res"] = res
    out = np.empty((B, C, H, W), np.float32)
    k = 0
    for b in range(B):
        for half in range(2):
            out[b, :, half * HS:(half + 1) * HS, :] = res.results[k]["out"]
            k += 1
    return out


if __name__ == "__main__":
    rng = np.random.default_rng(0)
    x = rng.standard_normal((B, C, H, W), dtype=np.float32)
    persp = rng.random((B, 1, H, W), dtype=np.float32)
    o = kernel(x=x, perspective=persp, alpha=np.ones(1, np.float32) * 3,
               beta=np.ones(1, np.float32), gamma=np.zeros(1, np.float32),
               kernel_size=7)
    print(o.shape, o.dtype, float(np.abs(o).mean()))


# revision 25
# speedup vs baseline: 1.0503x; 1.0065x over previous
"""Adaptive per-pixel Gaussian smoothing (7x7, sigma from a sigmoid of a
perspective map) on 8 Trainium2 NeuronCores — fp16 datapath revision.

Same structure as the fp32 baseline (data-parallel over (batch, H-half);
channel-major SBUF layout, ring-sum decomposition into 10 distinct-weight
rings) with three throughput changes:

1. fp16 everywhere on the DVE path (x, ring sums, weight maps, products):
   tensor_tensor runs in 2x_1p mode (2 elem/cycle/lane) instead of fp32 1x.
   To keep every operand 4B-aligned (required for 2x_1p), the x slab is
   DMA'd twice at column parities 0/1 (xwA/xwB) so every +-1/+-2/+-3 column
   shift reads from an even element offset. PSUM accumulation stays fp32.
2. The 10 weighted products are fused to one DVE op per ring (FD=2048).
3. Engine rebalance with a one-slab software pipeline: GpSimd (Pool)
   computes the three column sums R_b for slab s+1 while DVE runs slab
   s's rings and products (Pool's inputs are DMA-ready long in advance,
   and its outputs aren't needed until the next slab, so Pool stays off
   the critical path; slab 0's R_b run on the otherwise-idle DVE during
   the preamble). DMA issue lives on the sync (SP) sequencer; the
   PSUM->SBUF weight-broadcast copies are on ACT. Deep tile-pool
   buffering (xw 3, us/tm 4) decouples the DMA->DVE->PE->ACT pipeline;
   measured (cost model) DVE occupancy is ~88%.
"""

import numpy as np

import concourse.bass as bass
import concourse.tile as tile
from concourse import mybir
from concourse.bass_utils import run_bass_kernel_spmd

F32 = mybir.dt.float32
F16 = mybir.dt.float16
AF = mybir.ActivationFunctionType
OP = mybir.AluOpType

B, C, H, W = 4, 64, 256, 256
NCORES = 8
HS = H // 2          # 128 rows per core
G = 2                # row groups per core (partitions = G*64 channels)
GR = HS // G         # 64 rows per group
S = 8                # slab rows
NSLAB = GR // S      # 8 slabs
WP = W + 6           # 262 padded cols
WT = WP + 2          # 264-wide tiles so parity copy B fits at offset 1
LN2 = 0.6931471805599453

DS = [0, 1, 2, 4, 5, 8, 9, 10, 13, 18]
RING = {0: [(0, 0)], 1: [(0, 1), (1, 0)], 2: [(1, 1)], 4: [(0, 4), (4, 0)],
        5: [(1, 4), (4, 1)], 8: [(4, 4)], 9: [(0, 9), (9, 0)],
        10: [(1, 9), (9, 1)], 13: [(4, 9), (9, 4)], 18: [(9, 9)]}
POOL_D = ()          # ring chains all on DVE; Pool owns the column sums
DVE_D = [0, 4, 1, 8, 2, 5, 9, 10, 13, 18]
D_GROUPS = [(0,), (4, 1), (8, 2), (5, 9), (10, 18), (13,)]   # ordered by R_b availability
PIPE_AFTER = 3       # emit next slab's R_b/Pool work after this many DVE rings

_CACHE = {}


def _build_nc():
    nc = bass.Bass()
    x_in = nc.declare_dram_parameter("x", [C, GR * G + 6, WP], F16, isOutput=False)
    p_in = nc.declare_dram_parameter("persp", [HS, W], F32, isOutput=False)
    abg_in = nc.declare_dram_parameter("abg", [128, 3], F32, isOutput=False)
    sels_in = nc.declare_dram_parameter("sels", [128, GR, 128], F16, isOutput=False)
    id_in = nc.declare_dram_parameter("ident", [128, 128], F16, isOutput=False)
    out_d = nc.declare_dram_parameter("out", [C, HS, W], F32, isOutput=True)

    H2 = S // 2
    Q = H2 // 2

    with tile.TileContext(nc) as tc:
        with (
            tc.tile_pool(name="const", bufs=1) as constp,
            tc.tile_pool(name="maps", bufs=1) as mapsp,
            tc.tile_pool(name="xw", bufs=2) as xwp,
            tc.tile_pool(name="rr", bufs=2) as rrp,
            tc.tile_pool(name="cd", bufs=2) as cdp,
            tc.tile_pool(name="us", bufs=2) as usp,
            tc.tile_pool(name="tm", bufs=2) as tmp_,
            tc.tile_pool(name="ob", bufs=1) as obp,
            tc.tile_pool(name="ps", bufs=2, space="PSUM") as psp,
            tc.tile_pool(name="pso", bufs=1, space="PSUM") as psop,
        ):
            # ---------- preamble: constants ----------
            # persp/abg first: the 16KB/partition sels DMA would otherwise
            # delay the ACT weight-map chain by ~7us at startup
            abg = constp.tile([128, 3], F32, tag="abg", name="abg")
            nc.sync.dma_start(abg[:], abg_in[:])

            persp = mapsp.tile([128, W], F32, tag="persp", name="persp_sb")
            nc.sync.dma_start(persp[:], p_in[:])

            nln2 = constp.tile([128, 1], F32, tag="nln2", name="nln2")
            nc.gpsimd.memset(nln2[:], -LN2)

            # ---------- preamble: per-pixel weight maps (pixel-major) ----------

            def mtile(tag, dt=F32):
                return mapsp.tile([128, W], dt, tag=tag, name=tag)

            sg = mtile("sg")
            nc.scalar.activation(sg[:], persp[:], AF.Sigmoid,
                                 bias=abg[:, 2:3], scale=abg[:, 1:2])
            sig = mtile("sig")
            nc.vector.tensor_scalar(sig[:], sg[:], abg[:, 0:1], 1e-4,
                                    OP.mult, OP.max)
            lg = mtile("sg")
            nc.scalar.activation(lg[:], sig[:], AF.Ln)
            tt = mtile("sig")
            nc.scalar.activation(tt[:], lg[:], AF.Exp, bias=nln2[:], scale=-2.0)
            e = {}
            e[1] = mtile("e1")
            nc.scalar.activation(e[1][:], tt[:], AF.Exp, scale=-1.0)
            for d, (i, j) in ((2, (1, 1)), (4, (2, 2)), (5, (4, 1)), (8, (4, 4)),
                              (9, (8, 1)), (10, (8, 2)), (13, (9, 4)), (18, (9, 9))):
                e[d] = mtile(f"e{d}")
                nc.gpsimd.tensor_mul(e[d][:], e[i][:], e[j][:])
            ssum = mtile("ssum")
            nc.gpsimd.tensor_add(ssum[:], e[1][:], e[4][:])
            nc.gpsimd.tensor_add(ssum[:], ssum[:], e[9][:])
            sv = mtile("sv")
            nc.gpsimd.tensor_scalar(sv[:], ssum[:], 2.0, 1.0, OP.mult, OP.add)
            l2 = mtile("l2")
            nc.scalar.activation(l2[:], sv[:], AF.Ln)
            u0f = mtile("u0f")
            nc.scalar.activation(u0f[:], l2[:], AF.Exp, scale=-2.0)
            # fp16 weight maps for the broadcast matmuls
            u = {}
            u[0] = mtile("u0", F16)
            nc.scalar.copy(u[0][:], u0f[:])
            for d in DS[1:]:
                u[d] = mtile(f"u{d}", F16)
                nc.gpsimd.tensor_mul(u[d][:], e[d][:], u0f[:])

            # ---------- slab-level helpers (state keyed per slab) ----------
            def dma_slab(s):
                """DMA both column-parity copies of the padded slab: image col
                c sits at col c+3 in xwA and c+4 in xwB so every shifted read
                lands on an even fp16 element (4B-aligned -> DVE 2x_1p)."""
                xwA = xwp.tile([128, S + 6, WT], F16, tag="xwA", name="xwA")
                xwB = xwp.tile([128, S + 6, WT], F16, tag="xwB", name="xwB")
                for g in range(G):
                    src = x_in[:, g * GR + s * S: g * GR + s * S + S + 6, :]
                    nc.sync.dma_start(xwA[64 * g:64 * (g + 1), :, 0:WP], src)
                    nc.sync.dma_start(xwB[64 * g:64 * (g + 1), :, 1:1 + WP], src)
                return xwA, xwB

            def col_sums(xwA, xwB, eng=None):
                """Symmetric column sums R_b[c] = x[c-r] + x[c+r] (DVE), valid
                cols 0..255, rows covering the +-3 halo."""
                Reng = eng or nc.gpsimd
                R = {}
                # ordered by first consumer (d4/d8 then d1 then d9)
                R[4] = rrp.tile([128, S + 6, W], F16, tag="R4", name="R4")
                Reng.tensor_add(R[4][:], xwB[:, :, 2:2 + W], xwB[:, :, 6:6 + W])
                R[1] = rrp.tile([128, S + 6, W], F16, tag="R1", name="R1")
                Reng.tensor_add(R[1][:], xwA[:, :, 2:2 + W], xwA[:, :, 4:4 + W])
                R[9] = rrp.tile([128, S + 6, W], F16, tag="R9", name="R9")
                Reng.tensor_add(R[9][:], xwA[:, :, 0:W], xwA[:, :, 6:6 + W])
                return R

            def center(xwB, rows):   # unshifted columns, row slice of the slab
                return xwB[:, rows, 4:4 + W]

            def pair_sum(d, R, tag):
                """The second (a,b) pair of ring d as a standalone add; its
                inputs are this slab's R_b (ready at slab start) and its
                consumer is the late cd combine, so it can run on Pool in the
                window before the next slab's column sums."""
                (a, b) = RING[d][1]
                ra = int(np.sqrt(a))
                tb = cdp.tile([128, S, W], F16, tag=tag, name=tag, bufs=2)
                nc.gpsimd.tensor_add(tb[:], R[b][:, 3 - ra:3 - ra + S, :],
                                     R[b][:, 3 + ra:3 + ra + S, :])
                return tb

            def ring_sum(d, R, xwB, eng, tag, bufs=1, tb_pre=None,
                         into=None):
                """fp16 ring sum C_d computed on `eng`; returns the AP.
                `into=(tile, row0)` writes into a shared paired tile."""
                if d == 0:
                    return center(xwB, slice(3, 3 + S))
                if into is not None:
                    big, r0 = into
                    cd = big[:, r0:r0 + S, :]
                else:
                    cd = cdp.tile([128, S, W], F16, tag=tag, name=tag,
                                  bufs=bufs)
                first = True
                pend = None
                for pi, (a, b) in enumerate(RING[d]):
                    ra = int(np.sqrt(a))
                    if a == 0:
                        pend = R[b][:, 3:3 + S, :]
                        continue
                    if tb_pre is not None and pi == 1:
                        eng.tensor_add(cd[:], cd[:], tb_pre[:])
                        continue
                    hi = R[b][:, 3 - ra:3 - ra + S, :] if b else \
                        center(xwB, slice(3 - ra, 3 - ra + S))
                    lo = R[b][:, 3 + ra:3 + ra + S, :] if b else \
                        center(xwB, slice(3 + ra, 3 + ra + S))
                    if first:
                        eng.tensor_add(cd[:], hi, lo)
                        first = False
                    else:
                        tb = cdp.tile([128, S, W], F16, tag=f"{tag}_t",
                                      name=f"{tag}_t")
                        eng.tensor_add(tb[:], hi, lo)
                        eng.tensor_add(cd[:], cd[:], tb[:])
                if pend is not None:
                    eng.tensor_add(cd[:], cd[:], pend)
                return cd[:]

            def broadcast_us(s, d, tag, into=None):
                """u_d broadcast across the 128 (group, channel) partitions
                via selector matmuls into PSUM; ACT copies to fp16 SBUF.
                `into=(tile, row0)` writes into a shared paired tile."""
                if into is not None:
                    big, r0 = into
                    us = big[:, r0:r0 + S, :]
                else:
                    us = usp.tile([128, S, W], F16, tag=tag, name=tag)
                for h in range(2):
                    ur = psp.tile([128, H2, W], F32, tag="urep", name="urep")
                    for r2 in range(H2):
                        row = s * S + h * H2 + r2
                        nc.tensor.matmul(ur[:, r2, :], sels[:, row, :],
                                         u[d][:], start=True, stop=True)
                    nc.scalar.copy(us[:, h * H2:(h + 1) * H2, :], ur[:])
                return us

            def emit_pool_side(s, R, xwB):
                """Next-slab Pool work: the d=5/13 ring chains (GpSimd)."""
                return {d: ring_sum(d, R, xwB, nc.gpsimd, f"cdP{d}", bufs=2)
                        for d in POOL_D}

            # ---------- prologue: slab 0's inputs and Pool-side work ----------
            xw = dma_slab(0)
            Rc = col_sums(*xw, eng=nc.vector)
            tbP = {d2: pair_sum(d2, Rc, f"tbP{d2}") for d2 in (13,)}

            # broadcast constants: not needed until the first selector matmul
            sels = constp.tile([128, GR, 128], F16, tag="sels", name="sels")
            nc.sync.dma_start(sels[:], sels_in[:])
            ident = constp.tile([128, 128], F16, tag="ident", name="ident")
            nc.sync.dma_start(ident[:], id_in[:])
            usP = {d: broadcast_us(0, d, f"usP{d}") for d in POOL_D}
            cdP = emit_pool_side(0, Rc, xw[1])

            # ---------- main loop over slabs ----------
            for s in range(NSLAB):
                xwA, xwB = xw
                R = Rc

                oacc = [psop.tile([128, H2, W], F32, tag=f"oacc{h}",
                                  name=f"oacc{h}") for h in range(2)]

                # d-sum accumulated by PE identity matmuls into PSUM. Products
                # are queued and emitted one ring late so the in-order PE queue
                # never head-of-line blocks on the DVE/Pool product it consumes.
                started = [False, False]
                pend_acc = []

                def flush_acc(last=False):
                    for k, (tm, nparts) in enumerate(pend_acc):
                        is_last_tm = last and k == len(pend_acc) - 1
                        for p in range(nparts):
                            last_p = is_last_tm and p == nparts - 1
                            for h in range(2):
                                for q in range(2):
                                    rows = slice(p * S + h * H2 + q * Q,
                                                 p * S + h * H2 + (q + 1) * Q)
                                    nc.tensor.matmul(
                                        oacc[h][:, q * Q:(q + 1) * Q, :],
                                        ident[:],
                                        tm[:, rows, :],
                                        start=not started[h],
                                        stop=last_p and q == 1,
                                        skip_group_check=True)
                                started[h] = True
                    pend_acc.clear()

                this_usP, this_cdP = usP, cdP

                late_acc = []
                this_tbP = tbP
                for di, grp in enumerate(D_GROUPS):
                    flush_acc()
                    if len(grp) == 1:
                        d = grp[0]
                        us = broadcast_us(s, d, "us")
                        cd_ap = ring_sum(d, R, xwB, nc.vector, "cd",
                                         tb_pre=this_tbP.get(d))
                        tm = tmp_.tile([128, S, W], F16, tag="tm", name="tm")
                        nc.vector.tensor_mul(tm[:], cd_ap, us[:])
                        pend_acc.append((tm, 1))
                    else:
                        # paired rings: shared 16-row cd/us tiles, ONE product
                        # (halves the per-instruction dispatch/wait overhead)
                        us2 = usp.tile([128, 2 * S, W], F16, tag="us2",
                                       name="us2", bufs=3)
                        cd2 = cdp.tile([128, 2 * S, W], F16, tag="cd2",
                                       name="cd2", bufs=1)
                        for p, d in enumerate(grp):
                            broadcast_us(s, d, "us", into=(us2, p * S))
                            ring_sum(d, R, xwB, nc.vector, "cd",
                                     tb_pre=this_tbP.get(d),
                                     into=(cd2, p * S))
                        tm2 = tmp_.tile([128, 2 * S, W], F16, tag="tm2",
                                        name="tm2", bufs=2)
                        nc.vector.tensor_mul(tm2[:], cd2[:], us2[:])
                        pend_acc.append((tm2, 2))

                    if di == PIPE_AFTER and s + 1 < NSLAB:
                        # pipeline: next slab's inputs, column sums, and Pool
                        # chains are emitted here so Pool's work straddles the
                        # slab boundary and stays off the critical path
                        xw = dma_slab(s + 1)
                        Rc = col_sums(*xw)
                        usP = {d2: broadcast_us(s + 1, d2, f"usP{d2}")
                               for d2 in POOL_D}
                        cdP = emit_pool_side(s + 1, Rc, xw[1])
                        tbP = {d2: pair_sum(d2, Rc, f"tbP{d2}")
                               for d2 in (13,)}

                # Pool-side ring products on DVE at the end (chains long done)
                for d in POOL_D:
                    tmx = tmp_.tile([128, S, W], F16, tag=f"tmP{d}",
                                    name=f"tmP{d}")
                    nc.vector.tensor_mul(tmx[:], this_cdP[d], this_usP[d][:])
                    pend_acc.append((tmx, 1))
                pend_acc.extend((t, 1) for t in late_acc)
                flush_acc(last=True)

                out_sb = obp.tile([128, S, W], F32, tag="ob", name="ob")
                for h in range(2):
                    nc.scalar.copy(out_sb[:, h * H2:(h + 1) * H2, :],
                                   oacc[h][:])
                # per-(g,h) output DMA: the h=0 half ships while PE/ACT are
                # still finishing h=1, shortening the final-slab tail
                for g in range(G):
                    for h in range(2):
                        r0 = s * S + h * H2
                        nc.sync.dma_start(
                            out_d[:, g * GR + r0: g * GR + r0 + H2, :],
                            out_sb[64 * g:64 * (g + 1),
                                   h * H2:(h + 1) * H2, :])
    return nc


def _selectors():
    """sels[k, i, m] = 1 iff pixel-row k feeds out partition m at row index i."""
    if "sels" not in _CACHE:
        sels = np.zeros((128, GR, 128), np.float16)
        for i in range(GR):
            sels[i, i, 0:64] = 1.0          # group 0: pixel row i
            sels[GR + i, i, 64:128] = 1.0   # group 1: pixel row 64+i
        _CACHE["sels"] = sels
    return _CACHE["sels"]


def _split_waits(nc):
    """Walrus on this toolchain accepts only one semaphore wait per compute
    instruction; hoist excess waits onto same-engine NoOps placed before."""
    for f in nc.m.functions:
        for bb in f.blocks:
            new_list = []
            for ins in bb.instructions:
                si = ins.sync_info
                if si is not None and len(si.on_wait) > 1:
                    waits = list(si.on_wait)
                    for k, w in enumerate(waits[:-1]):
                        nop = mybir.InstNoOp(name=f"{ins.name}-ws{k}",
                                             ins=[], outs=[])
                        nop.engine = ins.engine
                        nop.sync_info = mybir.SyncInfo(on_wait=[w], on_update=[])
                        new_list.append(nop)
                    ins.sync_info = mybir.SyncInfo(on_wait=[waits[-1]],
                                                  on_update=list(si.on_update))
                new_list.append(ins)
            bb.instructions = new_list


def _get_nc():
    if "nc" not in _CACHE:
        nc = _build_nc()
        _split_waits(nc)
        _CACHE["nc"] = nc
    return _CACHE["nc"]


def kernel(x, perspective, alpha, beta, gamma, kernel_size):
    assert int(kernel_size) == 7
    x = np.asarray(x, dtype=np.float32)
    perspective = np.asarray(perspective, dtype=np.float32)
    a = np.float32(np.asarray(alpha).reshape(-1)[0])
    bt = np.float32(np.asarray(beta).reshape(-1)[0])
    gm = np.float32(np.asarray(gamma).reshape(-1)[0])
    abg = np.broadcast_to(np.array([a, bt, gm], np.float32), (128, 3)).copy()
    sels = _selectors()
    ident = np.eye(128, dtype=np.float16)

    xp = np.pad(x.astype(np.float16), ((0, 0), (0, 0), (3, 3), (3, 3)))
    in_maps = []
    for b in range(B):
        for half in range(2):
            r0 = half * HS
            in_maps.append({
                "x": np.ascontiguousarray(xp[b, :, r0:r0 + HS + 6, :]),
                "persp": np.ascontiguousarray(perspective[b, 0, r0:r0 + HS, :]),
                "abg": abg,
                "sels": sels,
                "ident": ident,
            })

    nc = _get_nc()
    res = run_bass_kernel_spmd(nc, in_maps, list(range(NCORES)))
    _CACHE["last_res"] = res
    out = np.empty((B, C, H, W), np.float32)
    k = 0
    for b in range(B):
        for half in range(2):
            out[b, :, half * HS:(half + 1) * HS, :] = res.results[k]["out"]
            k += 1
    return out


if __name__ == "__main__":
    rng = np.random.default_rng(0)
    x = rng.standard_normal((B, C, H, W), dtype=np.float32)
    persp = rng.random((B, 1, H, W), dtype=np.float32)
    o = kernel(x=x, perspective=persp, alpha=np.ones(1, np.float32) * 3,
               beta=np.ones(1, np.float32), gamma=np.zeros(1, np.float32),
               kernel_size=7)
    print(o.shape, o.dtype, float(np.abs(o).mean()))
